# revision 1
# baseline (speedup 1.0000x reference)
"""Trainium2 Bass kernel for nn_BlockConv (PointNet-style GNN block), 8 cores.

Algebraic core: msg_e = concat(x_src, pos_src-pos_dst) @ W + b
  = A[src] - C[dst], with A = concat(x,pos)@W + b (per-node table) and
  C = pos@W[-3:] (per-dst, constant within a segment). segment_max over
  dst therefore = (gather+max of A rows) - C[dst]. Pure memory problem.

Distribution: dst-sharded. Core k owns dst nodes [k*NLOC,(k+1)*NLOC);
node tensors and weights replicated; h exchanged with one AllGather
(transposed so conv2 needs no on-chip transposes); BN stats via tiny
AllReduduce of per-core partial sums.

Gather: dma_gather (int16 idx) from an HBM A-table (row n+1 = A[n],
row 0 = -BIG lo-dummy, rows N+1.. = -BIG incl. hi-dummy). Edges split
into lo/hi source windows to fit int16; per window, the core's nodes
are sorted by degree so pass k covers a slot prefix; DVE max chains
accumulate; a final HBM round-trip regathers both accumulators in node
order and maxes them.
"""
import sys
import numpy as np

if "/opt/trn_rl_repo" not in sys.path:
    sys.path.insert(0, "/opt/trn_rl_repo")

BIG_NEG = -1.0e30
EPS = 1e-5

FULL_CFG = dict(N=50000, E=800000, CIN=64, COUT=128, NC=8,
                LO_LIM=32768, R=50432, HI_DUMMY=50176)
MINI_CFG = dict(N=2048, E=16384, CIN=64, COUT=128, NC=8,
                LO_LIM=1024, R=2432, HI_DUMMY=2176)
MID_CFG = dict(N=16384, E=262144, CIN=64, COUT=128, NC=8,
               LO_LIM=8192, R=16768, HI_DUMMY=16512)


def _ceil(a, b):
    return (a + b - 1) // b


def _wrap16(ids):
    """flat int list (len % 128 == 0) -> [128, len//16] int16 wrapped:
    unwrapped[j] = g[j%16, j//16], replicated over the 8 core groups."""
    a = np.asarray(ids, np.int64)
    assert a.size % 128 == 0 and a.min() >= 0 and a.max() < 32768
    g = a.reshape(a.size // 16, 16).T.astype(np.int16)   # [16, L/16]
    return np.tile(g, (8, 1))                            # [128, L/16]


def host_prep(edge_index, pos, cfg):
    N, NC, LO_LIM = cfg["N"], cfg["NC"], cfg["LO_LIM"]
    NLOC = N // NC
    SLOC = _ceil(NLOC, 128)
    NSLOT = SLOC * 128
    src = np.asarray(edge_index[0], np.int64)
    dst = np.asarray(edge_index[1], np.int64)
    rows = src + 1
    core_of = dst // NLOC

    sides = [[], []]     # sides[0][c] = lo side of core c
    for c in range(NC):
        m = core_of == c
        s_rows = rows[m]
        d_loc = dst[m] - c * NLOC
        for si, sel in ((0, s_rows < LO_LIM), (1, s_rows >= LO_LIM)):
            s = s_rows[sel] - (0 if si == 0 else LO_LIM)
            d = d_loc[sel]
            deg = np.bincount(d, minlength=NSLOT)
            order = np.argsort(-deg, kind="stable")
            slot_of = np.empty(NSLOT, np.int64)
            slot_of[order] = np.arange(NSLOT)
            isort = np.argsort(d, kind="stable")
            starts = np.zeros(NSLOT + 1, np.int64)
            np.cumsum(deg, out=starts[1:])
            sides[si].append({"deg": deg, "order": order, "slot_of": slot_of,
                              "s_sorted": s[isort], "starts": starts,
                              "cnts": np.sort(deg)[::-1]})

    sched = []
    for si in range(2):
        Sk = []
        kmax = max(int(sd["cnts"][0]) for sd in sides[si])
        for k in range(kmax):
            cnt = max(int((sd["cnts"] > k).sum()) for sd in sides[si])
            if cnt == 0:
                break
            Sk.append(_ceil(cnt, 128))
        sched.append(Sk)

    j = np.arange(NSLOT)
    n_of_j = (j % 128) * SLOC + j // 128

    # interleaved to match the 512-node build blocks: column b*512+k*128+p
    # holds pos of node b*512+4p+k
    q = np.arange(_ceil(N, 512) * 512)
    node_q = np.minimum((q // 512) * 512 + (q % 128) * 4 + (q % 512) // 128, N - 1)
    posT = np.ascontiguousarray(np.asarray(pos)[node_q].T.astype(np.float32))

    per_core = []
    for c in range(NC):
        blocks = {0: [], 1: []}
        for si in range(2):
            sd = sides[si][c]
            dummy = 0 if si == 0 else cfg["HI_DUMMY"] - LO_LIM
            for k, S in enumerate(sched[si]):
                L = S * 128
                ids = np.full(L, dummy, np.int64)
                nsl = int((sd["cnts"] > k).sum())
                nodes = sd["order"][:nsl]
                ids[:nsl] = sd["s_sorted"][sd["starts"][nodes] + k]
                blocks[si].append(_wrap16(ids))
        gi_lo = (np.concatenate(blocks[0], axis=1) if blocks[0]
                 else np.zeros((128, 8), np.int16))
        gi_hi = (np.concatenate(blocks[1], axis=1) if blocks[1]
                 else np.zeros((128, 8), np.int16))
        mg_lo = _wrap16(sides[0][c]["slot_of"][n_of_j])
        mg_hi = _wrap16(sides[1][c]["slot_of"][n_of_j] + NSLOT)
        mg_sk = _wrap16(n_of_j)
        gnode = np.minimum(c * NLOC + n_of_j, N - 1)
        posm = np.ascontiguousarray(np.asarray(pos)[gnode].T.astype(np.float32))
        per_core.append({"gi_lo": gi_lo, "gi_hi": gi_hi, "mg_lo": mg_lo,
                         "mg_hi": mg_hi, "mg_skip": mg_sk, "posm": posm})

    # conv2 window pos permutation (global, replicated):
    win = np.arange(NC * SLOC)
    cols = []
    for w in win:
        ct, ww = w // SLOC, w % SLOC
        nodes = ct * NLOC + np.arange(128) * SLOC + ww
        cols.append(np.minimum(nodes, N - 1))
    posw = np.ascontiguousarray(
        np.asarray(pos)[np.concatenate(cols)].T.astype(np.float32))
    return per_core, (posw, posT), sched


def build_bass(cfg, sched, reps=1, timeline=False):
    import concourse.bass as bass
    import concourse.bacc as bacc
    import concourse.tile as tile
    from concourse import mybir
    from concourse.masks import make_identity
    import contextlib

    N, NC = cfg["N"], cfg["NC"]
    CIN, COUT = cfg["CIN"], cfg["COUT"]
    NLOC = N // NC
    SLOC = _ceil(NLOC, 128)
    NSLOT = SLOC * 128
    LO_LIM, R = cfg["LO_LIM"], cfg["R"]
    NCHUNK = _ceil(N, 128)
    NSK = _ceil(NLOC, 128)
    f32, i16 = mybir.dt.float32, mybir.dt.int16
    OP = mybir.AluOpType
    AX = mybir.AxisListType
    AF = mybir.ActivationFunctionType

    nc = bacc.Bacc(num_devices=(1 if timeline else NC), name="blockconv")

    x_in = nc.dram_tensor("x", [N, CIN], f32, kind="ExternalInput")
    pos_in = nc.dram_tensor("pos", [N, 3], f32, kind="ExternalInput")
    xs_in = nc.dram_tensor("xs", [NLOC, CIN], f32, kind="ExternalInput")
    posm_in = nc.dram_tensor("posm", [3, NSLOT], f32, kind="ExternalInput")
    posw_in = nc.dram_tensor("posw", [3, NC * NSLOT], f32, kind="ExternalInput")
    posT_in = nc.dram_tensor("posT", [3, _ceil(N, 512) * 512], f32, kind="ExternalInput")
    wt = {}
    for nm, shp in (("W1", [CIN + 3, COUT]), ("b1", [1, COUT]),
                    ("W2", [COUT + 3, COUT]), ("b2", [1, COUT]),
                    ("Wl", [CIN, COUT]), ("bl", [1, COUT]),
                    ("g1", [COUT, 1]), ("be1", [COUT, 1]), ("g2", [COUT, 1]),
                    ("be2", [COUT, 1]), ("gl", [COUT, 1]), ("bel", [COUT, 1])):
        wt[nm] = nc.dram_tensor(nm, shp, f32, kind="ExternalInput")

    Wlo = max(sum(sched[0]), 1) * 8
    Whi = max(sum(sched[1]), 1) * 8
    gi_lo_in = nc.dram_tensor("gi_lo", [128, Wlo], i16, kind="ExternalInput")
    gi_hi_in = nc.dram_tensor("gi_hi", [128, Whi], i16, kind="ExternalInput")
    mg_lo_in = nc.dram_tensor("mg_lo", [128, NSLOT // 16], i16, kind="ExternalInput")
    mg_hi_in = nc.dram_tensor("mg_hi", [128, NSLOT // 16], i16, kind="ExternalInput")
    mg_sk_in = nc.dram_tensor("mg_skip", [128, NSLOT // 16], i16, kind="ExternalInput")

    out_t = nc.dram_tensor("out", [NSLOT, COUT], f32, kind="ExternalOutput")

    HI_R = R - LO_LIM
    table_lo = nc.dram_tensor("atable_lo", [LO_LIM + SLOC * 128 + 128, COUT], f32)
    table_hi = nc.dram_tensor("atable_hi", [HI_R + SLOC * 128 + 128, COUT], f32)
    mbuf = nc.dram_tensor("mbuf", [2 * NSLOT, COUT], f32)
    skipb = nc.dram_tensor("skipbuf", [NSLOT, COUT], f32)
    ag_i = nc.dram_tensor("ag_in", [COUT, NSLOT], f32)
    ag_o = nc.dram_tensor("ag_out", [NC, COUT, NSLOT], f32, addr_space=("Local" if timeline else "Shared"))
    ar_i = nc.dram_tensor("ar_in", [COUT, 4], f32)
    ar_o = nc.dram_tensor("ar_out", [COUT, 4], f32, addr_space=("Local" if timeline else "Shared"))
    rowbuf = nc.dram_tensor("rowbuf", [6, COUT], f32)
    ar2_i = nc.dram_tensor("ar2_in", [COUT, 2], f32)
    ar2_o = nc.dram_tensor("ar2_out", [COUT, 2], f32, addr_space=("Local" if timeline else "Shared"))
    groups = [list(range(NC))]

    with tile.TileContext(nc) as tc:
        ctx = contextlib.ExitStack()
        with ctx:
            sing = ctx.enter_context(tc.tile_pool(name="sing", bufs=1))
            xp = ctx.enter_context(tc.tile_pool(name="xp", bufs=3))
            pp = ctx.enter_context(tc.tile_pool(name="pp", bufs=2, space="PSUM"))
            pq = ctx.enter_context(tc.tile_pool(name="pq", bufs=2, space="PSUM"))
            pr = ctx.enter_context(tc.tile_pool(name="pr", bufs=1, space="PSUM"))
            cp = ctx.enter_context(tc.tile_pool(name="cp", bufs=4))
            ap_ = ctx.enter_context(tc.tile_pool(name="ap", bufs=1))
            st = ctx.enter_context(tc.tile_pool(name="st", bufs=2))
            sm = ctx.enter_context(tc.tile_pool(name="sm", bufs=2))

            ident = sing.tile([128, 128], f32)
            make_identity(nc, ident)
            ones1 = sing.tile([1, 128], f32)
            nc.vector.memset(ones1[:], 1.0)
            onesp = sing.tile([128, 1], f32)
            nc.vector.memset(onesp[:], 1.0)
            negbig = sing.tile([128, COUT], f32)
            nc.vector.memset(negbig[:], BIG_NEG)
            epsv = sing.tile([COUT, 1], f32)
            nc.vector.memset(epsv[:], EPS)

            W1s = sing.tile([CIN + 3, COUT], f32)
            nc.sync.dma_start(W1s[:], wt["W1"][:])
            W1ps = sing.tile([3, COUT], f32)
            nc.sync.dma_start(W1ps[:], wt["W1"][CIN:CIN + 3, :])
            W2as = sing.tile([COUT, COUT], f32)
            nc.sync.dma_start(W2as[:], wt["W2"][0:COUT, :])
            W2ps = sing.tile([3, COUT], f32)
            nc.sync.dma_start(W2ps[:], wt["W2"][COUT:COUT + 3, :])
            Wls = sing.tile([CIN, COUT], f32)
            nc.sync.dma_start(Wls[:], wt["Wl"][:])
            brow = {}
            for nm in ("b1", "b2", "bl"):
                t = sing.tile([1, COUT], f32, tag=f"br_{nm}")
                nc.sync.dma_start(t[:], wt[nm][:])
                brow[nm] = t
            b1bc = sing.tile([128, COUT], f32)
            _b1ap = wt["b1"][:]
            nc.sync.dma_start(b1bc[:], bass.AP(tensor=_b1ap.tensor, offset=_b1ap.offset,
                                               ap=[[0, 128]] + list(_b1ap.ap[1:])))
            pvec = {}
            for nm in ("g1", "be1", "g2", "be2", "gl", "bel"):
                v = sing.tile([COUT, 1], f32, tag=f"pv_{nm}")
                nc.sync.dma_start(v[:], wt[nm][:])
                pvec[nm] = v

            idx_lo = sing.tile([128, Wlo], i16)
            nc.sync.dma_start(idx_lo[:], gi_lo_in[:])
            idx_hi = sing.tile([128, Whi], i16)
            nc.sync.dma_start(idx_hi[:], gi_hi_in[:])
            midx = {}
            for nm, t_ in (("lo", mg_lo_in), ("hi", mg_hi_in), ("sk", mg_sk_in)):
                m_ = sing.tile([128, NSLOT // 16], i16, tag=f"mi_{nm}")
                nc.sync.dma_start(m_[:], t_[:])
                midx[nm] = m_

            for _rep in range(reps):
                # -BIG rows: lo dummy row 0; hi rows N+1..R
                nc.sync.dma_start(table_lo[0:1, :], negbig[0:1, :])
                r = N + 1
                while r < R:
                    nn = min(128, R - r)
                    nc.sync.dma_start(table_hi[r - LO_LIM:r - LO_LIM + nn, :], negbig[0:nn, :])
                    r += nn

                def table_write(src_tile, base, nrows):
                    lo_n = max(0, min(LO_LIM - base, nrows))
                    if lo_n > 0:
                        nc.sync.dma_start(table_lo[base:base + lo_n, :], src_tile[0:lo_n, :])
                    if lo_n < nrows:
                        hb = base + lo_n - LO_LIM
                        nc.sync.dma_start(table_hi[hb:hb + nrows - lo_n, :],
                                          src_tile[lo_n:nrows, :])

                # ---------------- conv1 A-table build ----------------
                # 512-node blocks: partition p holds rows base+4p..+3 (1KB
                # contiguous per partition); 4 interleaved transposes; table
                # rows written with stride 4.
                def stride4_write(src_tile, A, mlim):
                    m0 = max(0, min(mlim, _ceil(LO_LIM - A, 4)))
                    if m0 > 0:
                        d = table_lo[A:A + m0 * 4, :].rearrange(
                            "(m s) f -> m s f", s=4)[:, 0, :]
                        nc.sync.dma_start(d, src_tile[0:m0, :])
                    if m0 < mlim:
                        b2 = A + m0 * 4 - LO_LIM
                        d = table_hi[b2:b2 + (mlim - m0) * 4, :].rearrange(
                            "(m s) f -> m s f", s=4)[:, 0, :]
                        nc.sync.dma_start(d, src_tile[m0:mlim, :])

                NBLK = _ceil(N, 512)
                for b in range(NBLK):
                    base = b * 512
                    nload = min(512, N - base)
                    pmax = nload // 4
                    xt4 = xp.tile([128, 4, CIN], f32, tag="xload")
                    nc.sync.dma_start(
                        xt4[:pmax],
                        x_in[base:base + nload, :].rearrange("(p r) c -> p r c", r=4))
                    for k in range(4):
                        mlim_k = max(0, min(128, _ceil(N - base - k, 4)))
                        if mlim_k == 0:
                            continue
                        ps = pp.tile([CIN, 128], f32, tag="pst")
                        nc.tensor.transpose(out=ps[:], in_=xt4[:, k, :], identity=ident[:])
                        lhs = xp.tile([CIN + 3, 128], f32, tag="lhs")
                        nc.sync.dma_start(lhs[CIN:CIN + 3, :],
                                          posT_in[:, b * 512 + k * 128:b * 512 + (k + 1) * 128])
                        nc.scalar.copy(out=lhs[0:CIN, :], in_=ps[:])
                        pb = pq.tile([128, COUT], f32, tag="pout")
                        nc.tensor.matmul(out=pb[:], lhsT=lhs[:], rhs=W1s[:], start=True, stop=False)
                        nc.tensor.matmul(out=pb[:], lhsT=ones1[:], rhs=brow["b1"][:], start=False, stop=True)
                        oc = cp.tile([128, COUT], f32, tag="oc")
                        nc.vector.tensor_copy(out=oc[:], in_=pb[:])
                        stride4_write(oc, 1 + base + k, mlim_k)

                # ---------------- skip path (x slice @ Wl + bl) ----------------
                sk_s = sm.tile([128, COUT], f32, tag="sk_s")
                sk_q = sm.tile([128, COUT], f32, tag="sk_q")
                nc.vector.memset(sk_s[:], 0.0)
                nc.vector.memset(sk_q[:], 0.0)
                for c in range(NSK):
                    r0 = c * 128
                    nrow = min(128, NLOC - r0)
                    xt = xp.tile([128, CIN], f32, tag="xload")
                    nc.sync.dma_start(xt[:nrow, :], xs_in[r0:r0 + nrow, :])
                    ps = pp.tile([CIN, 128], f32, tag="pst")
                    nc.tensor.transpose(out=ps[:], in_=xt[:], identity=ident[:])
                    lhs = xp.tile([CIN + 3, 128], f32, tag="lhs")
                    nc.scalar.copy(out=lhs[0:CIN, :], in_=ps[:])
                    pb = pq.tile([128, COUT], f32, tag="pout")
                    nc.tensor.matmul(out=pb[:], lhsT=lhs[0:CIN, :], rhs=Wls[:], start=True, stop=False)
                    nc.tensor.matmul(out=pb[:], lhsT=ones1[:], rhs=brow["bl"][:], start=False, stop=True)
                    oc = cp.tile([128, COUT], f32, tag="oc")
                    nc.vector.tensor_copy(out=oc[:], in_=pb[:])
                    nc.sync.dma_start(skipb[r0:r0 + 128, :], oc[:])
                    nc.vector.tensor_tensor(out=sk_s[:nrow, :], in0=sk_s[:nrow, :], in1=oc[:nrow, :], op=OP.add)
                    sq = cp.tile([128, COUT], f32, tag="sq")
                    nc.vector.tensor_tensor(out=sq[:nrow, :], in0=oc[:nrow, :], in1=oc[:nrow, :], op=OP.mult)
                    nc.vector.tensor_tensor(out=sk_q[:nrow, :], in0=sk_q[:nrow, :], in1=sq[:nrow, :], op=OP.add)

                # stats staging tile [COUT, 4]: cols 0,1 conv1 sum/sq; 2,3 skip
                arst = sing.tile([COUT, 4], f32)
                pss = pr.tile([COUT, 2], f32, tag="pstat")
                nc.tensor.matmul(out=pss[:, 0:1], lhsT=sk_s[:], rhs=onesp[:], start=True, stop=True)
                nc.tensor.matmul(out=pss[:, 1:2], lhsT=sk_q[:], rhs=onesp[:], start=True, stop=True)
                nc.vector.tensor_copy(out=arst[:, 2:4], in_=pss[:])

                # ---------------- gather-max passes ----------------
                GMAX = 8   # max 8*128=1024 indices per dma_gather (HW SWDGE ring cap)

                def gather_chunked(dst3, in_ap, idxt, chunk0, nchunks):
                    a = 0
                    while a < nchunks:
                        b = min(a + GMAX, nchunks)
                        nc.gpsimd.dma_gather(
                            out_ap=dst3[:, a:b, :], in_ap=in_ap,
                            idxs_ap=idxt[:, (chunk0 + a) * 8:(chunk0 + b) * 8],
                            num_idxs=(b - a) * 128, num_idxs_reg=(b - a) * 128,
                            elem_size=COUT)
                        a = b

                def gather_conv(conv_idx):
                    acc = {}
                    for snm in ("lo", "hi"):
                        a = ap_.tile([128, SLOC, COUT], f32, tag=f"acc_{snm}")
                        nc.gpsimd.memset(a[:], BIG_NEG)
                        acc[snm] = a
                    for snm, idxt, wtab, winsz in (
                            ("lo", idx_lo, table_lo, LO_LIM), ("hi", idx_hi, table_hi, HI_R)):
                        off = 0
                        for k, S in enumerate(sched[0 if snm == "lo" else 1]):
                            stg = st.tile([128, SLOC, COUT], f32, tag="stage")
                            gather_chunked(stg[:, 0:S, :], wtab[0:winsz, :],
                                           idxt, off // 8, S)
                            nc.vector.tensor_tensor(
                                out=acc[snm][:, 0:S, :], in0=acc[snm][:, 0:S, :],
                                in1=stg[:, 0:S, :], op=OP.max)
                            off += 8 * S
                    # merge via HBM round-trip, node order
                    nc.sync.dma_start(
                        mbuf[0:NSLOT, :].rearrange("(s p) f -> p s f", p=128), acc["lo"][:])
                    nc.sync.dma_start(
                        mbuf[NSLOT:2 * NSLOT, :].rearrange("(s p) f -> p s f", p=128), acc["hi"][:])
                    g1t = st.tile([128, SLOC, COUT], f32, tag="stage")
                    gather_chunked(g1t[:], mbuf[:], midx["lo"], 0, SLOC)
                    g2t = st.tile([128, SLOC, COUT], f32, tag="stage")
                    gather_chunked(g2t[:], mbuf[:], midx["hi"], 0, SLOC)
                    agg = ap_.tile([128, SLOC, COUT], f32, tag="acc_lo")
                    nc.vector.tensor_tensor(out=agg[:], in0=g1t[:], in1=g2t[:], op=OP.max)
                    return agg

                agg1 = gather_conv(1)

                # mask = (agg1 > -1e29): 1.0 / 0.0  (deg-0 slots; reused for conv2)
                mask = sing.tile([128, SLOC, COUT], f32)
                nc.vector.tensor_scalar(out=mask[:], in0=agg1[:], scalar1=-1.0e29,
                                        scalar2=None, op0=OP.is_gt)

                # v1 = (agg1 - (c1 - b1)) * mask, per chunk s
                v1 = ap_.tile([128, SLOC, COUT], f32, tag="acc_hi")
                for s in range(SLOC):
                    pm = xp.tile([3, 128], f32, tag="posm")
                    nc.sync.dma_start(pm[:], posm_in[:, s * 128:(s + 1) * 128])
                    pc = pq.tile([128, COUT], f32, tag="pout")
                    nc.tensor.matmul(out=pc[:], lhsT=pm[:], rhs=W1ps[:], start=True, stop=True)
                    cb = cp.tile([128, COUT], f32, tag="cb")
                    nc.vector.tensor_tensor(out=cb[:], in0=pc[:], in1=b1bc[:], op=OP.subtract)
                    t_ = cp.tile([128, COUT], f32, tag="tv")
                    nc.vector.tensor_tensor(out=t_[:], in0=agg1[:, s, :], in1=cb[:], op=OP.subtract)
                    nc.vector.tensor_tensor(out=v1[:, s, :], in0=t_[:], in1=mask[:, s, :], op=OP.mult)

                # conv1 stats over v1
                def stats_into(vtile, arcols):
                    red = sm.tile([128, COUT], f32, tag="red")
                    nc.vector.tensor_reduce(out=red[:], in_=vtile[:].rearrange("p s f -> p f s"),
                                            op=OP.add, axis=AX.X)
                    vsq = st.tile([128, SLOC, COUT], f32, tag="stage")
                    nc.vector.tensor_tensor(out=vsq[:], in0=vtile[:], in1=vtile[:], op=OP.mult)
                    redq = sm.tile([128, COUT], f32, tag="redq")
                    nc.vector.tensor_reduce(out=redq[:], in_=vsq[:].rearrange("p s f -> p f s"),
                                            op=OP.add, axis=AX.X)
                    pst_ = pr.tile([COUT, 2], f32, tag="pstat")
                    nc.tensor.matmul(out=pst_[:, 0:1], lhsT=red[:], rhs=onesp[:], start=True, stop=True)
                    nc.tensor.matmul(out=pst_[:, 1:2], lhsT=redq[:], rhs=onesp[:], start=True, stop=True)
                    nc.vector.tensor_copy(out=arcols, in_=pst_[:])

                stats_into(v1, arst[:, 0:2])
                nc.sync.dma_start(ar_i[:], arst[:])
                if timeline:
                    _t = sm.tile([COUT, 4], f32, tag="cc1")
                    nc.sync.dma_start(_t[:], ar_i[:])
                    nc.sync.dma_start(ar_o[:], _t[:])
                else:
                    nc.gpsimd.collective_compute("AllReduce", OP.add, replica_groups=groups,
                                                 ins=[ar_i[:]], outs=[ar_o[:]])
                arres = sing.tile([COUT, 4], f32, tag="arres")
                nc.sync.dma_start(arres[:], ar_o[:])

                # BN params: scale = g * rsqrt(var+eps), shift = be - mean*scale
                def bn_params(sum_ap, sq_ap, g_v, be_v, tagp):
                    mean = sm.tile([COUT, 1], f32, tag=f"{tagp}_m")
                    nc.vector.tensor_scalar(out=mean[:], in0=sum_ap, scalar1=1.0 / N,
                                            scalar2=None, op0=OP.mult)
                    ex2 = sm.tile([COUT, 1], f32, tag=f"{tagp}_e")
                    nc.vector.tensor_scalar(out=ex2[:], in0=sq_ap, scalar1=1.0 / N,
                                            scalar2=None, op0=OP.mult)
                    m2 = sm.tile([COUT, 1], f32, tag=f"{tagp}_m2")
                    nc.vector.tensor_tensor(out=m2[:], in0=mean[:], in1=mean[:], op=OP.mult)
                    var = sm.tile([COUT, 1], f32, tag=f"{tagp}_v")
                    nc.vector.tensor_tensor(out=var[:], in0=ex2[:], in1=m2[:], op=OP.subtract)
                    sd = sm.tile([COUT, 1], f32, tag=f"{tagp}_sd")
                    nc.scalar.activation(out=sd[:], in_=var[:], func=AF.Sqrt, bias=epsv[:], scale=1.0)
                    rstd = sm.tile([COUT, 1], f32, tag=f"{tagp}_r")
                    nc.vector.reciprocal(out=rstd[:], in_=sd[:])
                    ssh = sm.tile([COUT, 2], f32, tag=f"{tagp}_ssh")
                    nc.vector.tensor_tensor(out=ssh[:, 0:1], in0=rstd[:], in1=g_v[:], op=OP.mult)
                    ms = sm.tile([COUT, 1], f32, tag=f"{tagp}_ms")
                    nc.vector.tensor_tensor(out=ms[:], in0=mean[:], in1=ssh[:, 0:1], op=OP.mult)
                    nc.vector.tensor_tensor(out=ssh[:, 1:2], in0=be_v[:], in1=ms[:], op=OP.subtract)
                    # transpose [COUT,2] -> [2, COUT] rows (scale row 0, shift row 1)
                    prow = pr.tile([2, COUT], f32, tag="prow")
                    nc.tensor.transpose(out=prow[:], in_=ssh[:], identity=ident[:])
                    rows = sing.tile([2, COUT], f32, tag=f"{tagp}_rows")
                    nc.vector.tensor_copy(out=rows[:], in_=prow[:])
                    slot = {"bn1": 0, "bnl": 2, "bn2": 4}[tagp]
                    nc.sync.dma_start(rowbuf[slot:slot + 2, :], rows[:])
                    bc = sing.tile([128, 2, COUT], f32, tag=f"{tagp}_bc")
                    rap = rowbuf[slot:slot + 2, :]
                    nc.sync.dma_start(bc[:], bass.AP(tensor=rap.tensor, offset=rap.offset,
                                                     ap=[[0, 128]] + list(rap.ap)))
                    return bc

                rows1 = bn_params(arres[:, 0:1], arres[:, 1:2], pvec["g1"], pvec["be1"], "bn1")
                rowsl = bn_params(arres[:, 2:3], arres[:, 3:4], pvec["gl"], pvec["bel"], "bnl")

                # h = relu(v1*scale1 + shift1); build transposed hT chunks -> ag_in
                h1 = ap_.tile([128, SLOC, COUT], f32, tag="acc_hi2")
                sc3 = rows1[:, 0:1, :].to_broadcast([128, SLOC, COUT])
                sh3 = rows1[:, 1:2, :].to_broadcast([128, SLOC, COUT])
                nc.vector.tensor_tensor(out=h1[:], in0=v1[:], in1=sc3, op=OP.mult)
                nc.vector.tensor_tensor(out=h1[:], in0=h1[:], in1=sh3, op=OP.add)
                nc.vector.tensor_scalar(out=h1[:], in0=h1[:], scalar1=0.0, scalar2=None, op0=OP.max)
                for s in range(SLOC):
                    ph = pq.tile([128, 128], f32, tag="pout")
                    nc.tensor.transpose(out=ph[:], in_=h1[:, s, :], identity=ident[:])
                    hc = cp.tile([128, 128], f32, tag="oc")
                    nc.scalar.copy(out=hc[:], in_=ph[:])
                    nc.sync.dma_start(ag_i[:, s * 128:(s + 1) * 128], hc[:])

                if timeline:
                    for _s in range(SLOC):
                        _t = cp.tile([128, 128], f32, tag="oc")
                        nc.sync.dma_start(_t[:], ag_i[:, _s * 128:(_s + 1) * 128])
                        nc.sync.dma_start(ag_o[0, :, _s * 128:(_s + 1) * 128], _t[:])
                else:
                    nc.gpsimd.collective_compute("AllGather", OP.bypass, replica_groups=groups,
                                                 ins=[ag_i[:]], outs=[ag_o[:]])

                # ---------------- conv2 A-table build (no transposes) ----------
                for ct in range(NC):
                    for w in range(SLOC):
                        win = ct * SLOC + w
                        lhs = xp.tile([COUT, 128], f32, tag="lhs2")
                        nc.sync.dma_start(lhs[:], ag_o[ct, :, w * 128:(w + 1) * 128])
                        pm = xp.tile([3, 128], f32, tag="posm")
                        nc.sync.dma_start(pm[:], posw_in[:, win * 128:(win + 1) * 128])
                        pb = pq.tile([128, COUT], f32, tag="pout")
                        nc.tensor.matmul(out=pb[:], lhsT=lhs[:], rhs=W2as[:], start=True, stop=False)
                        nc.tensor.matmul(out=pb[:], lhsT=pm[:], rhs=W2ps[:], start=False, stop=False)
                        nc.tensor.matmul(out=pb[:], lhsT=ones1[:], rhs=brow["b2"][:], start=False, stop=True)
                        oc = cp.tile([128, COUT], f32, tag="oc")
                        nc.vector.tensor_copy(out=oc[:], in_=pb[:])
                        mlim = min(128, _ceil(NLOC - w, SLOC))
                        base = 1 + ct * NLOC + w
                        m0 = max(0, min(mlim, _ceil(LO_LIM - base, SLOC)))
                        if m0 > 0:
                            d = table_lo[base:base + m0 * SLOC, :].rearrange(
                                "(m s) f -> m s f", s=SLOC)[:, 0, :]
                            nc.sync.dma_start(d, oc[0:m0, :])
                        if m0 < mlim:
                            b2 = base + m0 * SLOC - LO_LIM
                            d = table_hi[b2:b2 + (mlim - m0) * SLOC, :].rearrange(
                                "(m s) f -> m s f", s=SLOC)[:, 0, :]
                            nc.sync.dma_start(d, oc[m0:mlim, :])

                agg2 = gather_conv(2)

                # v2 = (agg2 - c2) * mask
                v2 = ap_.tile([128, SLOC, COUT], f32, tag="acc_hi")
                for s in range(SLOC):
                    pm = xp.tile([3, 128], f32, tag="posm")
                    nc.sync.dma_start(pm[:], posm_in[:, s * 128:(s + 1) * 128])
                    pc = pq.tile([128, COUT], f32, tag="pout")
                    nc.tensor.matmul(out=pc[:], lhsT=pm[:], rhs=W2ps[:], start=True, stop=True)
                    t_ = cp.tile([128, COUT], f32, tag="tv")
                    nc.vector.tensor_tensor(out=t_[:], in0=agg2[:, s, :], in1=pc[:], op=OP.subtract)
                    nc.vector.tensor_tensor(out=v2[:, s, :], in0=t_[:], in1=mask[:, s, :], op=OP.mult)

                arst2 = sing.tile([COUT, 2], f32, tag="arst2")
                stats_into(v2, arst2[:])
                nc.sync.dma_start(ar2_i[:], arst2[:])
                if timeline:
                    _t = sm.tile([COUT, 2], f32, tag="cc2")
                    nc.sync.dma_start(_t[:], ar2_i[:])
                    nc.sync.dma_start(ar2_o[:], _t[:])
                else:
                    nc.gpsimd.collective_compute("AllReduce", OP.add, replica_groups=groups,
                                                 ins=[ar2_i[:]], outs=[ar2_o[:]])
                arres2 = sing.tile([COUT, 2], f32, tag="arres2")
                nc.sync.dma_start(arres2[:], ar2_o[:])
                rows2 = bn_params(arres2[:, 0:1], arres2[:, 1:2], pvec["g2"], pvec["be2"], "bn2")

                # final = relu(bn2(v2) + bnl(skip))
                skg = st.tile([128, SLOC, COUT], f32, tag="stage")
                gather_chunked(skg[:], skipb[:], midx["sk"], 0, SLOC)
                fin = ap_.tile([128, SLOC, COUT], f32, tag="acc_hi2")
                nc.vector.tensor_tensor(out=fin[:], in0=v2[:],
                                        in1=rows2[:, 0:1, :].to_broadcast([128, SLOC, COUT]), op=OP.mult)
                nc.vector.tensor_tensor(out=fin[:], in0=fin[:],
                                        in1=rows2[:, 1:2, :].to_broadcast([128, SLOC, COUT]), op=OP.add)
                skbn = st.tile([128, SLOC, COUT], f32, tag="stage")
                nc.vector.tensor_tensor(out=skbn[:], in0=skg[:],
                                        in1=rowsl[:, 0:1, :].to_broadcast([128, SLOC, COUT]), op=OP.mult)
                nc.vector.tensor_tensor(out=skbn[:], in0=skbn[:],
                                        in1=rowsl[:, 1:2, :].to_broadcast([128, SLOC, COUT]), op=OP.add)
                nc.vector.tensor_tensor(out=fin[:], in0=fin[:], in1=skbn[:], op=OP.add)
                nc.vector.tensor_scalar(out=fin[:], in0=fin[:], scalar1=0.0, scalar2=None, op0=OP.max)
                nc.sync.dma_start(out_t[:].rearrange("(p s) f -> p s f", p=128), fin[:])

    nc.compile()
    return nc


def make_in_maps(inputs, cfg, per_core, posw):
    posw, posT = posw
    N, NC, CIN = cfg["N"], cfg["NC"], cfg["CIN"]
    NLOC = N // NC
    shared = dict(
        posT=posT,
        x=np.ascontiguousarray(np.asarray(inputs["x"], np.float32)),
        pos=np.ascontiguousarray(np.asarray(inputs["pos"], np.float32)),
        posw=posw,
        W1=np.asarray(inputs["W1"], np.float32),
        b1=np.asarray(inputs["b1"], np.float32).reshape(1, -1),
        W2=np.asarray(inputs["W2"], np.float32),
        b2=np.asarray(inputs["b2"], np.float32).reshape(1, -1),
        Wl=np.asarray(inputs["Wl"], np.float32),
        bl=np.asarray(inputs["bl"], np.float32).reshape(1, -1),
        g1=np.asarray(inputs["g1"], np.float32).reshape(-1, 1),
        be1=np.asarray(inputs["be1"], np.float32).reshape(-1, 1),
        g2=np.asarray(inputs["g2"], np.float32).reshape(-1, 1),
        be2=np.asarray(inputs["be2"], np.float32).reshape(-1, 1),
        gl=np.asarray(inputs["gl"], np.float32).reshape(-1, 1),
        bel=np.asarray(inputs["bel"], np.float32).reshape(-1, 1),
    )
    in_maps = []
    for c in range(NC):
        m = dict(shared)
        m["xs"] = np.ascontiguousarray(shared["x"][c * NLOC:(c + 1) * NLOC])
        pc = per_core[c]
        m["gi_lo"] = pc["gi_lo"]
        m["gi_hi"] = pc["gi_hi"]
        m["mg_lo"] = pc["mg_lo"]
        m["mg_hi"] = pc["mg_hi"]
        m["mg_skip"] = pc["mg_skip"]
        m["posm"] = pc["posm"]
        in_maps.append(m)
    return in_maps


_CACHE = {}


def run(inputs, cfg, use_sim=False, trace=False):
    per_core, posw, sched = host_prep(inputs["edge_index"], inputs["pos"], cfg)
    key = (cfg["N"], tuple(sched[0]), tuple(sched[1]))
    if key not in _CACHE:
        _CACHE[key] = build_bass(cfg, sched)
    nc = _CACHE[key]
    in_maps = make_in_maps(inputs, cfg, per_core, posw)
    NC = cfg["NC"]
    NLOC = cfg["N"] // NC
    if use_sim:
        from concourse.bass_interp import MultiCoreSim
        sim = MultiCoreSim(nc, num_cores=NC, require_finite=False, require_nnan=False)
        for c in range(NC):
            for k, v in in_maps[c].items():
                sim.cores[c].tensor(k)[:] = v
        sim.simulate(check_with_hw=False)
        outs = [np.array(sim.cores[c].tensor("out")) for c in range(NC)]
        res = None
    else:
        from concourse.bass_utils import run_bass_kernel_spmd
        res = run_bass_kernel_spmd(nc, in_maps, core_ids=list(range(NC)), trace=trace)
        outs = [res.results[c]["out"] for c in range(NC)]
    full = np.concatenate([o[:NLOC] for o in outs], axis=0)
    return full, res


def kernel(**inputs):
    out, _ = run(inputs, FULL_CFG, use_sim=False)
    return out



# revision 5
# speedup vs baseline: 3.0308x; 3.0308x over previous
"""Trainium2 Bass kernel for nn_BlockConv (PointNet-style GNN block), 8 cores.

Algebraic core: msg_e = concat(x_src, pos_src-pos_dst) @ W + b
  = A[src] - C[dst], with A = concat(x,pos)@W (per-node table, bias folded
  into C) and C = pos@W[-3:] - b (per-dst, constant within a segment).
  segment_max over dst = (gather+max of A rows) - C[dst]. Memory-bound.

Distribution: dst-sharded; per-core edge gathers from a replicated fp16
A-table in HBM via 4-queue SWDGE dma_gather (descriptor-rate limited:
~3.5ns/row on 4 queues vs 8.8 on one). Tables, stages and h are fp16
(tolerance 2e-2 >> fp16 rounding). Host pre-transposes x/pos so table
builds are straight fp16 matmuls (no on-chip transposes); dst slots are
degree-sorted per side (lo/hi source windows for int16 idx) so gather
pass k covers a slot prefix. Per conv: DVE max-accumulates into an acc
tile, the hi side is re-gathered into lo-slot order via one extra HBM
round-trip, PE transposes give the feature-major layout for BN
(DVE-reduce stats, tiny AllReduce), and h^T feeds an fp16 AllGather that
conv2's table build consumes directly. Output is feature-major in
lo2-slot order; the host unpermutes.
"""
import sys
import numpy as np

if "/opt/trn_rl_repo" not in sys.path:
    sys.path.insert(0, "/opt/trn_rl_repo")

BIG_NEG = -60000.0
MASK_THR = -30000.0
EPS = 1e-5

FULL_CFG = dict(N=50000, E=800000, CIN=64, COUT=128, NC=8,
                LO_LIM=32768, R=50432)
MINI_CFG = dict(N=2048, E=16384, CIN=64, COUT=128, NC=8,
                LO_LIM=1024, R=2432)
MID_CFG = dict(N=16384, E=262144, CIN=64, COUT=128, NC=8,
               LO_LIM=8192, R=16768)


def _ceil(a, b):
    return (a + b - 1) // b


def _wrap16(ids):
    """flat int list (len % 128 == 0) -> [128, len//16] int16 wrapped:
    unwrapped[j] = g[j%16, j//16], replicated over the 8 core groups."""
    a = np.asarray(ids, np.int64)
    assert a.size % 128 == 0 and a.min() >= 0 and a.max() < 32768
    g = a.reshape(a.size // 16, 16).T.astype(np.int16)   # [16, L/16]
    return np.tile(g, (8, 1))                            # [128, L/16]


def _side_prep(rows, d_loc, NSLOT):
    """Degree-sort dst slots for one source window. rows = side-local table
    row ids per edge; d_loc = local dst node position per edge."""
    deg = np.bincount(d_loc, minlength=NSLOT)
    order = np.argsort(-deg, kind="stable")        # slot s -> node position
    slot_of = np.empty(NSLOT, np.int64)            # node position -> slot
    slot_of[order] = np.arange(NSLOT)
    isort = np.argsort(d_loc, kind="stable")
    starts = np.zeros(NSLOT + 1, np.int64)
    np.cumsum(deg, out=starts[1:])
    return {"deg": deg, "order": order, "slot_of": slot_of,
            "s_sorted": rows[isort], "starts": starts,
            "cnts": np.sort(deg)[::-1]}


def _build_sched(sides_by_core):
    """sched[si] = list of S_k (128-slot chunks) per pass, max across cores."""
    sched = []
    for si in range(2):
        Sk = []
        kmax = max(int(sd[si]["cnts"][0]) for sd in sides_by_core)
        for k in range(kmax):
            cnt = max(int((sd[si]["cnts"] > k).sum()) for sd in sides_by_core)
            if cnt == 0:
                break
            Sk.append(_ceil(cnt, 128))
        sched.append(Sk)
    return sched


def _build_gi(side, sched_side, dummy):
    blocks = []
    for k, S in enumerate(sched_side):
        L = S * 128
        ids = np.full(L, dummy, np.int64)
        nsl = int((side["cnts"] > k).sum())
        nodes = side["order"][:nsl]
        ids[:nsl] = side["s_sorted"][side["starts"][nodes] + k]
        blocks.append(_wrap16(ids))
    if not blocks:
        return np.zeros((128, 8), np.int16)
    return np.concatenate(blocks, axis=1)


def host_prep(edge_index, pos, x, cfg):
    N, NC, LO_LIM, R = cfg["N"], cfg["NC"], cfg["LO_LIM"], cfg["R"]
    NLOC = N // NC
    SLOC = _ceil(NLOC, 128)
    NSLOT = SLOC * 128
    NPAD1 = _ceil(N, 1024) * 1024
    HID = R - 1 - LO_LIM          # hi-local dummy row (global row R-1)
    src = np.asarray(edge_index[0], np.int64)
    dst = np.asarray(edge_index[1], np.int64)
    core_of = dst // NLOC
    pos = np.asarray(pos, np.float32)
    x = np.asarray(x, np.float32)

    # ---- conv1: per-core lo/hi sides in source-node space ----
    data1, slot1_lo_glob = [], np.empty(N, np.int64)
    for c in range(NC):
        m = core_of == c
        r = src[m] + 1
        d = dst[m] - c * NLOC
        sides = []
        for si, sel in ((0, r < LO_LIM), (1, r >= LO_LIM)):
            sides.append(_side_prep(r[sel] - (LO_LIM if si else 0),
                                    d[sel], NSLOT))
        data1.append(sides)
        inv = sides[0]["slot_of"][:NLOC]
        slot1_lo_glob[c * NLOC:(c + 1) * NLOC] = inv
    sched1 = _build_sched(data1)

    # ---- conv2: rows in (core, lo1-slot) space ----
    r2 = 1 + (src // NLOC) * NSLOT + slot1_lo_glob[src]
    data2 = []
    for c in range(NC):
        m = core_of == c
        r = r2[m]
        d = dst[m] - c * NLOC
        sides = []
        for si, sel in ((0, r < LO_LIM), (1, r >= LO_LIM)):
            sides.append(_side_prep(r[sel] - (LO_LIM if si else 0),
                                    d[sel], NSLOT))
        data2.append(sides)
    sched2 = _build_sched(data2)

    # ---- per-core tensors ----
    per_core = []
    for c in range(NC):
        s1, s2 = data1[c], data2[c]
        node1 = np.minimum(c * NLOC + s1[0]["order"], N - 1)   # lo1 slot -> node
        node2 = np.minimum(c * NLOC + s2[0]["order"], N - 1)   # lo2 slot -> node
        posm1 = np.empty((4, NSLOT), np.float16)
        posm1[:3] = pos[node1].T
        posm1[3] = -1.0
        posm2 = np.empty((4, NSLOT), np.float16)
        posm2[:3] = pos[node2].T
        posm2[3] = -1.0
        xs2 = np.zeros((64, NSLOT), np.float16)
        real2 = s2[0]["order"] < NLOC
        xs2[:, real2] = x[c * NLOC + s2[0]["order"][real2]].T
        per_core.append({
            "gi1_lo": _build_gi(s1[0], sched1[0], 0),
            "gi1_hi": _build_gi(s1[1], sched1[1], HID),
            "gi2_lo": _build_gi(s2[0], sched2[0], 0),
            "gi2_hi": _build_gi(s2[1], sched2[1], HID),
            "mg_hi1": _wrap16(s1[1]["slot_of"][s1[0]["order"]]),
            "mg_hi2": _wrap16(s2[1]["slot_of"][s2[0]["order"]]),
            "posm1": np.ascontiguousarray(posm1),
            "posm2": np.ascontiguousarray(posm2),
            "xsT2": np.ascontiguousarray(xs2),
            "order2_lo": s2[0]["order"],
        })

    # ---- shared tensors ----
    xpT = np.zeros((cfg["CIN"] + 3, NPAD1), np.float16)
    xpT[:cfg["CIN"], :N] = x.T
    xpT[cfg["CIN"]:, :N] = pos.T
    posw = np.empty((3, NC * NSLOT), np.float16)
    for c in range(NC):
        node1 = np.minimum(c * NLOC + data1[c][0]["order"], N - 1)
        posw[:, c * NSLOT:(c + 1) * NSLOT] = pos[node1].T
    shared = {"xpT": np.ascontiguousarray(xpT),
              "posw": np.ascontiguousarray(posw)}
    return per_core, shared, (sched1, sched2)


def build_bass(cfg, scheds, reps=1):
    import concourse.bass as bass
    import concourse.bacc as bacc
    import concourse.tile as tile
    from concourse import mybir
    from concourse.masks import make_identity
    import contextlib

    N, NC = cfg["N"], cfg["NC"]
    CIN, COUT = cfg["CIN"], cfg["COUT"]
    NLOC = N // NC
    SLOC = _ceil(NLOC, 128)
    NSLOT = SLOC * 128
    LO_LIM, R = cfg["LO_LIM"], cfg["R"]
    HI_R = R - LO_LIM
    NPAD1 = _ceil(N, 1024) * 1024
    sched1, sched2 = scheds
    f32, f16, i16 = mybir.dt.float32, mybir.dt.float16, mybir.dt.int16
    OP = mybir.AluOpType
    AX = mybir.AxisListType
    AF = mybir.ActivationFunctionType
    VC = min(512, NSLOT)          # v-compute chunk (slots)

    nc = bacc.Bacc(num_devices=NC, name="blockconv2", num_swdge_queues=4)

    xpT_in = nc.dram_tensor("xpT", [CIN + 3, NPAD1], f16, kind="ExternalInput")
    posw_in = nc.dram_tensor("posw", [3, NC * NSLOT], f16, kind="ExternalInput")
    xsT2_in = nc.dram_tensor("xsT2", [CIN, NSLOT], f16, kind="ExternalInput")
    posm1_in = nc.dram_tensor("posm1", [4, NSLOT], f16, kind="ExternalInput")
    posm2_in = nc.dram_tensor("posm2", [4, NSLOT], f16, kind="ExternalInput")
    wt = {}
    for nm, shp, dt in (("W1s", [CIN + 3, COUT], f16), ("W1pa", [4, COUT], f16),
                        ("W2as", [COUT, COUT], f16), ("W2pa", [4, COUT], f16),
                        ("Wls", [CIN, COUT], f16),
                        ("g1", [COUT, 1], f32), ("be1", [COUT, 1], f32),
                        ("g2", [COUT, 1], f32), ("be2", [COUT, 1], f32),
                        ("gl", [COUT, 1], f32), ("bel", [COUT, 1], f32)):
        wt[nm] = nc.dram_tensor(nm, shp, dt, kind="ExternalInput")

    W1lo = max(sum(sched1[0]), 1) * 8
    W1hi = max(sum(sched1[1]), 1) * 8
    W2lo = max(sum(sched2[0]), 1) * 8
    W2hi = max(sum(sched2[1]), 1) * 8
    gi_in = {}
    for nm, w in (("gi1_lo", W1lo), ("gi1_hi", W1hi),
                  ("gi2_lo", W2lo), ("gi2_hi", W2hi)):
        gi_in[nm] = nc.dram_tensor(nm, [128, w], i16, kind="ExternalInput")
    mg1_in = nc.dram_tensor("mg_hi1", [128, NSLOT // 16], i16, kind="ExternalInput")
    mg2_in = nc.dram_tensor("mg_hi2", [128, NSLOT // 16], i16, kind="ExternalInput")

    out_t = nc.dram_tensor("out", [COUT, NSLOT], f32, kind="ExternalOutput")

    table_lo = nc.dram_tensor("table_lo", [LO_LIM, COUT], f16)
    table_hi = nc.dram_tensor("table_hi", [HI_R, COUT], f16)
    mbuf = nc.dram_tensor("mbuf", [NSLOT, COUT], f16)
    ag_i = nc.dram_tensor("ag_in", [COUT, NSLOT], f16)
    ag_o = nc.dram_tensor("ag_out", [NC, COUT, NSLOT], f16, addr_space="Shared")
    ar_i = nc.dram_tensor("ar_in", [COUT, 4], f32)
    ar_o = nc.dram_tensor("ar_out", [COUT, 4], f32, addr_space="Shared")
    ar2_i = nc.dram_tensor("ar2_in", [COUT, 2], f32)
    ar2_o = nc.dram_tensor("ar2_out", [COUT, 2], f32, addr_space="Shared")
    groups = [list(range(NC))]

    qctr = [0]

    def nextq():
        q = qctr[0] & 3
        qctr[0] += 1
        return q

    with tile.TileContext(nc) as tc:
        ctx = contextlib.ExitStack()
        with ctx:
            sing = ctx.enter_context(tc.tile_pool(name="sing", bufs=1))
            ld = ctx.enter_context(tc.tile_pool(name="ld", bufs=2))
            st = ctx.enter_context(tc.tile_pool(name="st", bufs=2))
            big = ctx.enter_context(tc.tile_pool(name="big", bufs=1))
            vv = ctx.enter_context(tc.tile_pool(name="vv", bufs=2))
            ck = ctx.enter_context(tc.tile_pool(name="ck", bufs=2))
            ppb = ctx.enter_context(tc.tile_pool(name="ppb", bufs=4, space="PSUM"))
            pcs = ctx.enter_context(tc.tile_pool(name="pcs", bufs=2, space="PSUM"))
            ppt = ctx.enter_context(tc.tile_pool(name="ppt", bufs=2, space="PSUM"))

            ident = sing.tile([128, 128], f16)
            make_identity(nc, ident)
            negbig = sing.tile([1, COUT], f16)
            nc.vector.memset(negbig[:], BIG_NEG)
            epsv = sing.tile([COUT, 1], f32)
            nc.vector.memset(epsv[:], EPS)

            W = {}
            for nm in ("W1s", "W1pa", "W2as", "W2pa", "Wls"):
                t = sing.tile(list(wt[nm].shape), f16, tag=f"w_{nm}")
                nc.sync.dma_start(t[:], wt[nm][:])
                W[nm] = t
            pvec = {}
            for nm in ("g1", "be1", "g2", "be2", "gl", "bel"):
                v = sing.tile([COUT, 1], f32, tag=f"pv_{nm}")
                nc.sync.dma_start(v[:], wt[nm][:])
                pvec[nm] = v

            mg1 = sing.tile([128, NSLOT // 16], i16, tag="mg1")
            nc.sync.dma_start(mg1[:], mg1_in[:])
            mg2 = sing.tile([128, NSLOT // 16], i16, tag="mg2")
            nc.sync.dma_start(mg2[:], mg2_in[:])

            def table_write(oc, base, G):
                """Write oc[:, 0:G, :] (G*128 rows) at table row `base`."""
                n = G * 128
                m0 = max(0, min(LO_LIM - base, n))
                if m0 > 0:
                    gf, rem = divmod(m0, 128)
                    if gf:
                        d = table_lo[base:base + gf * 128, :].rearrange(
                            "(g p) f -> p g f", p=128)
                        nc.sync.dma_start(d, oc[:, 0:gf, :])
                    if rem:
                        nc.sync.dma_start(
                            table_lo[base + gf * 128:base + m0, :],
                            oc[0:rem, gf, :])
                if m0 < n:
                    b2 = base + m0 - LO_LIM
                    gf, rem = divmod(m0, 128)
                    if rem:
                        nc.sync.dma_start(
                            table_hi[b2:b2 + (128 - rem), :],
                            oc[rem:128, gf, :])
                        b2 += 128 - rem
                        gf += 1
                    if gf < G:
                        d = table_hi[b2:b2 + (G - gf) * 128, :].rearrange(
                            "(g p) f -> p g f", p=128)
                        nc.sync.dma_start(d, oc[:, gf:G, :])

            def gathers(gi, sched_c, acc):
                """Max-accumulate gather passes for one conv into acc
                [128, 2*SLOC, COUT] f16 (lo ranks then hi ranks)."""
                for si, (nm_win, winsz) in enumerate(
                        ((table_lo, LO_LIM), (table_hi, HI_R))):
                    idxt, off = gi[si], 0
                    for k, S in enumerate(sched_c[si]):
                        stg = st.tile([128, SLOC, COUT], f16, tag="stage")
                        a = 0
                        while a < S:
                            b = min(a + 8, S)
                            nc.gpsimd.dma_gather(
                                out_ap=stg[:, a:b, :], in_ap=nm_win[0:winsz, :],
                                idxs_ap=idxt[:, off + a * 8:off + b * 8],
                                num_idxs=(b - a) * 128,
                                num_idxs_reg=(b - a) * 128,
                                elem_size=COUT, queue_num=nextq())
                            a = b
                        ro = si * SLOC
                        nc.vector.tensor_tensor(
                            out=acc[:, ro:ro + S, :], in0=acc[:, ro:ro + S, :],
                            in1=stg[:, 0:S, :], op=OP.max)
                        off += S * 8

            def merge_and_v(acc, mg, posm_t, Wp_aug, vT):
                """acc -> agg (lo order) -> transpose -> v^T = (agg^T-c')*mask.
                Returns (vT filled, sum [COUT,1], sumsq [COUT,1])."""
                # hi side -> mbuf -> regather in lo-slot order
                nc.sync.dma_start(
                    mbuf[:].rearrange("(s p) f -> p s f", p=128),
                    acc[:, SLOC:2 * SLOC, :])
                hi_lo = st.tile([128, SLOC, COUT], f16, tag="stage")
                a = 0
                while a < SLOC:
                    b = min(a + 8, SLOC)
                    nc.gpsimd.dma_gather(
                        out_ap=hi_lo[:, a:b, :], in_ap=mbuf[:, :],
                        idxs_ap=mg[:, a * 8:b * 8],
                        num_idxs=(b - a) * 128, num_idxs_reg=(b - a) * 128,
                        elem_size=COUT, queue_num=nextq())
                    a = b
                nc.vector.tensor_tensor(out=hi_lo[:], in0=hi_lo[:],
                                        in1=acc[:, 0:SLOC, :], op=OP.max)
                vsum = sing.tile([COUT, 1], f32, tag="vsum")
                vsq = sing.tile([COUT, 1], f32, tag="vsq")
                for j0 in range(0, NSLOT, VC):
                    nr = min(VC, NSLOT - j0) // 128
                    pt = ppt.tile([128, VC // 128, 128], f16, tag="pt")
                    for r in range(nr):
                        nc.tensor.transpose(out=pt[:, r, :],
                                            in_=hi_lo[:, j0 // 128 + r, :],
                                            identity=ident[:])
                    ptf = pt[:, 0:nr, :].rearrange("p a b -> p (a b)")
                    w = nr * 128
                    aggf = ck.tile([128, VC], f32, tag="aggf")
                    nc.scalar.copy(out=aggf[:, 0:w], in_=ptf)
                    cps = pcs.tile([128, VC], f32, tag="cps")
                    nc.tensor.matmul(out=cps[:, 0:w], lhsT=Wp_aug[:],
                                     rhs=posm_t[:, j0:j0 + w],
                                     start=True, stop=True)
                    mask = ck.tile([128, VC], f32, tag="mask")
                    nc.vector.tensor_scalar(out=mask[:, 0:w], in0=aggf[:, 0:w],
                                            scalar1=MASK_THR, scalar2=None,
                                            op0=OP.is_gt)
                    tmp = ck.tile([128, VC], f32, tag="tmp")
                    nc.vector.tensor_tensor(out=tmp[:, 0:w], in0=aggf[:, 0:w],
                                            in1=cps[:, 0:w], op=OP.subtract)
                    nc.vector.tensor_tensor(out=vT[:, j0:j0 + w],
                                            in0=tmp[:, 0:w], in1=mask[:, 0:w],
                                            op=OP.mult)
                    ps = ck.tile([COUT, 1], f32, tag="psum1")
                    nc.vector.tensor_reduce(out=ps[:], in_=vT[:, j0:j0 + w],
                                            op=OP.add, axis=AX.X)
                    sq = ck.tile([128, VC], f32, tag="sqc")
                    nc.vector.tensor_tensor(out=sq[:, 0:w], in0=vT[:, j0:j0 + w],
                                            in1=vT[:, j0:j0 + w], op=OP.mult)
                    pq = ck.tile([COUT, 1], f32, tag="psum2")
                    nc.vector.tensor_reduce(out=pq[:], in_=sq[:, 0:w],
                                            op=OP.add, axis=AX.X)
                    if j0 == 0:
                        nc.vector.tensor_copy(out=vsum[:], in_=ps[:])
                        nc.vector.tensor_copy(out=vsq[:], in_=pq[:])
                    else:
                        nc.vector.tensor_tensor(out=vsum[:], in0=vsum[:],
                                                in1=ps[:], op=OP.add)
                        nc.vector.tensor_tensor(out=vsq[:], in0=vsq[:],
                                                in1=pq[:], op=OP.add)
                return vsum, vsq

            def bn_params(sum_ap, sq_ap, g_v, be_v, tagp):
                """scale = g*rsqrt(var+eps), shift = be - mean*scale; [COUT,1]."""
                mean = ck.tile([COUT, 1], f32, tag=f"{tagp}_m")
                nc.vector.tensor_scalar(out=mean[:], in0=sum_ap, scalar1=1.0 / N,
                                        scalar2=None, op0=OP.mult)
                ex2 = ck.tile([COUT, 1], f32, tag=f"{tagp}_e")
                nc.vector.tensor_scalar(out=ex2[:], in0=sq_ap, scalar1=1.0 / N,
                                        scalar2=None, op0=OP.mult)
                m2 = ck.tile([COUT, 1], f32, tag=f"{tagp}_m2")
                nc.vector.tensor_tensor(out=m2[:], in0=mean[:], in1=mean[:],
                                        op=OP.mult)
                var = ck.tile([COUT, 1], f32, tag=f"{tagp}_v")
                nc.vector.tensor_tensor(out=var[:], in0=ex2[:], in1=m2[:],
                                        op=OP.subtract)
                sd = ck.tile([COUT, 1], f32, tag=f"{tagp}_sd")
                nc.scalar.activation(out=sd[:], in_=var[:], func=AF.Sqrt,
                                     bias=epsv[:], scale=1.0)
                rstd = ck.tile([COUT, 1], f32, tag=f"{tagp}_r")
                nc.vector.reciprocal(out=rstd[:], in_=sd[:])
                sc = sing.tile([COUT, 1], f32, tag=f"{tagp}_sc")
                nc.vector.tensor_tensor(out=sc[:], in0=rstd[:], in1=g_v[:],
                                        op=OP.mult)
                ms = ck.tile([COUT, 1], f32, tag=f"{tagp}_ms")
                nc.vector.tensor_tensor(out=ms[:], in0=mean[:], in1=sc[:],
                                        op=OP.mult)
                sh = sing.tile([COUT, 1], f32, tag=f"{tagp}_sh")
                nc.vector.tensor_tensor(out=sh[:], in0=be_v[:], in1=ms[:],
                                        op=OP.subtract)
                return sc, sh

            for _rep in range(reps):
                # ---- dummy rows ----
                nc.sync.dma_start(table_lo[0:1, :], negbig[:])
                nc.sync.dma_start(table_hi[HI_R - 1:HI_R, :], negbig[:])

                # ---- conv1 A-table build ----
                for c0 in range(0, NPAD1, 1024):
                    lhs = ld.tile([CIN + 3, 1024], f16, tag="lhs1")
                    nc.sync.dma_start(lhs[:], xpT_in[:, c0:c0 + 1024])
                    oc = ld.tile([128, 8, COUT], f16, tag="oc")
                    for g in range(8):
                        pb = ppb.tile([128, COUT], f32, tag="pb")
                        nc.tensor.matmul(out=pb[:],
                                         lhsT=lhs[:, g * 128:(g + 1) * 128],
                                         rhs=W["W1s"][:], start=True, stop=True)
                        nc.scalar.copy(out=oc[:, g, :], in_=pb[:])
                    table_write(oc, 1 + c0, 8)

                # ---- conv1 gathers ----
                gi1 = {}
                for si, nm in ((0, "gi1_lo"), (1, "gi1_hi")):
                    t = sing.tile([128, max(W1lo, W2lo) if si == 0
                                   else max(W1hi, W2hi)], i16,
                                  tag=f"gi_{si}")
                    nc.sync.dma_start(t[:, 0:gi_in[nm].shape[1]], gi_in[nm][:])
                    gi1[si] = t
                acc = big.tile([128, 2 * SLOC, COUT], f16, tag="acc")
                nc.vector.memset(acc[:], BIG_NEG)
                gathers(gi1, sched1, acc)

                # ---- skip path: skipT = Wl^T @ xsT2 (lo2 order) ----
                skipT = big.tile([COUT, NSLOT], f16, tag="skipT")
                sksum = sing.tile([COUT, 1], f32, tag="sksum")
                sksq = sing.tile([COUT, 1], f32, tag="sksq")
                for j0 in range(0, NSLOT, 1024):
                    w = min(1024, NSLOT - j0)
                    xs = ld.tile([CIN, 1024], f16, tag="lhs2")
                    nc.sync.dma_start(xs[:, 0:w], xsT2_in[:, j0:j0 + w])
                    for h0 in range(0, w, VC):
                        hw = min(VC, w - h0)
                        pskip = pcs.tile([128, VC], f32, tag="cps")
                        nc.tensor.matmul(out=pskip[:, 0:hw], lhsT=W["Wls"][:],
                                         rhs=xs[:, h0:h0 + hw],
                                         start=True, stop=True)
                        nc.scalar.copy(out=skipT[:, j0 + h0:j0 + h0 + hw],
                                       in_=pskip[:, 0:hw])
                        ps = ck.tile([COUT, 1], f32, tag="psum1")
                        nc.vector.tensor_reduce(out=ps[:], in_=pskip[:, 0:hw],
                                                op=OP.add, axis=AX.X)
                        sq = ck.tile([128, VC], f32, tag="sqc")
                        nc.vector.tensor_tensor(
                            out=sq[:, 0:hw],
                            in0=skipT[:, j0 + h0:j0 + h0 + hw],
                            in1=skipT[:, j0 + h0:j0 + h0 + hw], op=OP.mult)
                        pq = ck.tile([COUT, 1], f32, tag="psum2")
                        nc.vector.tensor_reduce(out=pq[:], in_=sq[:, 0:hw],
                                                op=OP.add, axis=AX.X)
                        if j0 == 0 and h0 == 0:
                            nc.vector.tensor_copy(out=sksum[:], in_=ps[:])
                            nc.vector.tensor_copy(out=sksq[:], in_=pq[:])
                        else:
                            nc.vector.tensor_tensor(out=sksum[:], in0=sksum[:],
                                                    in1=ps[:], op=OP.add)
                            nc.vector.tensor_tensor(out=sksq[:], in0=sksq[:],
                                                    in1=pq[:], op=OP.add)

                # ---- conv1 merge + v1 + stats ----
                posm1 = sing.tile([4, NSLOT], f16, tag="posm")
                nc.sync.dma_start(posm1[:], posm1_in[:])
                v1T = big.tile([COUT, NSLOT], f16, tag="vT")
                s1, q1 = merge_and_v(acc, mg1, posm1, W["W1pa"], v1T)

                arst = sing.tile([COUT, 4], f32, tag="arst")
                nc.vector.tensor_copy(out=arst[:, 0:1], in_=s1[:])
                nc.vector.tensor_copy(out=arst[:, 1:2], in_=q1[:])
                nc.vector.tensor_copy(out=arst[:, 2:3], in_=sksum[:])
                nc.vector.tensor_copy(out=arst[:, 3:4], in_=sksq[:])
                nc.sync.dma_start(ar_i[:], arst[:])
                nc.gpsimd.collective_compute(
                    "AllReduce", OP.add, replica_groups=groups,
                    ins=[ar_i[:]], outs=[ar_o[:]])
                arres = sing.tile([COUT, 4], f32, tag="arres")
                nc.sync.dma_start(arres[:], ar_o[:])

                sc1, sh1 = bn_params(arres[:, 0:1], arres[:, 1:2],
                                     pvec["g1"], pvec["be1"], "bn1")
                scl, shl = bn_params(arres[:, 2:3], arres[:, 3:4],
                                     pvec["gl"], pvec["bel"], "bnl")

                # h^T = relu(bn1(v1)) -> AllGather
                h1T = big.tile([COUT, NSLOT], f16, tag="h1T")
                nc.scalar.activation(out=h1T[:], in_=v1T[:], func=AF.Relu,
                                     bias=sh1[:], scale=sc1[:])
                nc.sync.dma_start(ag_i[:], h1T[:])
                nc.gpsimd.collective_compute(
                    "AllGather", OP.bypass, replica_groups=groups,
                    ins=[ag_i[:]], outs=[ag_o[:]])

                # ---- conv2 A-table build ----
                for ct in range(NC):
                    for c0 in range(0, NSLOT, 1024):
                        w = min(1024, NSLOT - c0)
                        nw = w // 128
                        lhs = ld.tile([COUT, 1024], f16, tag="lhs2b")
                        nc.sync.dma_start(lhs[:, 0:w], ag_o[ct, :, c0:c0 + w])
                        pw = ld.tile([3, 1024], f16, tag="posw")
                        nc.sync.dma_start(
                            pw[:, 0:w],
                            posw_in[:, ct * NSLOT + c0:ct * NSLOT + c0 + w])
                        oc = ld.tile([128, 8, COUT], f16, tag="oc")
                        for g in range(nw):
                            pb = ppb.tile([128, COUT], f32, tag="pb")
                            nc.tensor.matmul(out=pb[:],
                                             lhsT=lhs[:, g * 128:(g + 1) * 128],
                                             rhs=W["W2as"][:],
                                             start=True, stop=False)
                            nc.tensor.matmul(out=pb[:],
                                             lhsT=pw[:, g * 128:(g + 1) * 128],
                                             rhs=W["W2pa"][0:3, :],
                                             start=False, stop=True)
                            nc.scalar.copy(out=oc[:, g, :], in_=pb[:])
                        table_write(oc, 1 + ct * NSLOT + c0, nw)

                # ---- conv2 gathers ----
                gi2 = {}
                for si, nm in ((0, "gi2_lo"), (1, "gi2_hi")):
                    t = sing.tile([128, max(W1lo, W2lo) if si == 0
                                   else max(W1hi, W2hi)], i16,
                                  tag=f"gi_{si}")
                    nc.sync.dma_start(t[:, 0:gi_in[nm].shape[1]], gi_in[nm][:])
                    gi2[si] = t
                acc2 = big.tile([128, 2 * SLOC, COUT], f16, tag="acc")
                nc.vector.memset(acc2[:], BIG_NEG)
                gathers(gi2, sched2, acc2)

                # ---- conv2 merge + v2 + stats ----
                posm2 = sing.tile([4, NSLOT], f16, tag="posm")
                nc.sync.dma_start(posm2[:], posm2_in[:])
                v2T = big.tile([COUT, NSLOT], f16, tag="vT")
                s2, q2 = merge_and_v(acc2, mg2, posm2, W["W2pa"], v2T)

                arst2 = sing.tile([COUT, 2], f32, tag="arst2")
                nc.vector.tensor_copy(out=arst2[:, 0:1], in_=s2[:])
                nc.vector.tensor_copy(out=arst2[:, 1:2], in_=q2[:])
                nc.sync.dma_start(ar2_i[:], arst2[:])
                nc.gpsimd.collective_compute(
                    "AllReduce", OP.add, replica_groups=groups,
                    ins=[ar2_i[:]], outs=[ar2_o[:]])
                arres2 = sing.tile([COUT, 2], f32, tag="arres2")
                nc.sync.dma_start(arres2[:], ar2_o[:])
                sc2, sh2 = bn_params(arres2[:, 0:1], arres2[:, 1:2],
                                     pvec["g2"], pvec["be2"], "bn2")

                # ---- final = relu(bn2(v2) + bnl(skip)) ----
                for j0 in range(0, NSLOT, VC):
                    w = min(VC, NSLOT - j0)
                    a_ = ck.tile([128, VC], f32, tag="aggf")
                    nc.scalar.activation(out=a_[:, 0:w],
                                         in_=v2T[:, j0:j0 + w], func=AF.Identity,
                                         bias=sh2[:], scale=sc2[:])
                    b_ = ck.tile([128, VC], f32, tag="tmp")
                    nc.scalar.activation(out=b_[:, 0:w],
                                         in_=skipT[:, j0:j0 + w], func=AF.Identity,
                                         bias=shl[:], scale=scl[:])
                    fin = ck.tile([128, VC], f32, tag="mask")
                    nc.vector.tensor_tensor(out=fin[:, 0:w], in0=a_[:, 0:w],
                                            in1=b_[:, 0:w], op=OP.add)
                    nc.vector.tensor_scalar(out=fin[:, 0:w], in0=fin[:, 0:w],
                                            scalar1=0.0, scalar2=None,
                                            op0=OP.max)
                    nc.sync.dma_start(out_t[:, j0:j0 + w], fin[:, 0:w])

    nc.compile()
    return nc


def make_in_maps(inputs, cfg, per_core, shared):
    f16 = np.float16
    CIN, COUT = cfg["CIN"], cfg["COUT"]
    W1 = np.asarray(inputs["W1"], np.float32)
    b1 = np.asarray(inputs["b1"], np.float32)
    W2 = np.asarray(inputs["W2"], np.float32)
    b2 = np.asarray(inputs["b2"], np.float32)
    W1pa = np.concatenate([W1[CIN:CIN + 3, :], b1[None, :]], axis=0)
    W2pa = np.concatenate([W2[COUT:COUT + 3, :], b2[None, :]], axis=0)
    base = dict(
        xpT=shared["xpT"], posw=shared["posw"],
        W1s=W1.astype(f16), W1pa=W1pa.astype(f16),
        W2as=W2[0:COUT, :].astype(f16), W2pa=W2pa.astype(f16),
        Wls=np.asarray(inputs["Wl"], np.float32).astype(f16),
        g1=np.asarray(inputs["g1"], np.float32).reshape(-1, 1),
        be1=np.asarray(inputs["be1"], np.float32).reshape(-1, 1),
        g2=np.asarray(inputs["g2"], np.float32).reshape(-1, 1),
        be2=np.asarray(inputs["be2"], np.float32).reshape(-1, 1),
        gl=np.asarray(inputs["gl"], np.float32).reshape(-1, 1),
        bel=np.asarray(inputs["bel"], np.float32).reshape(-1, 1),
    )
    in_maps = []
    for pc in per_core:
        m = dict(base)
        for k in ("gi1_lo", "gi1_hi", "gi2_lo", "gi2_hi",
                  "mg_hi1", "mg_hi2", "posm1", "posm2", "xsT2"):
            m[k] = pc[k]
        in_maps.append(m)
    return in_maps


_CACHE = {}


def run(inputs, cfg, use_sim=False, trace=False):
    per_core, shared, scheds = host_prep(
        inputs["edge_index"], inputs["pos"], inputs["x"], cfg)
    key = (cfg["N"], tuple(scheds[0][0]), tuple(scheds[0][1]),
           tuple(scheds[1][0]), tuple(scheds[1][1]))
    if key not in _CACHE:
        _CACHE[key] = build_bass(cfg, scheds)
    nc = _CACHE[key]
    in_maps = make_in_maps(inputs, cfg, per_core, shared)
    NC = cfg["NC"]
    NLOC = cfg["N"] // NC
    if use_sim:
        from concourse.bass_interp import MultiCoreSim
        sim = MultiCoreSim(nc, num_cores=NC, require_finite=False,
                           require_nnan=False)
        for c in range(NC):
            for k, v in in_maps[c].items():
                sim.cores[c].tensor(k)[:] = v
        sim.simulate(check_with_hw=False)
        outs = [np.array(sim.cores[c].tensor("out")) for c in range(NC)]
        res = None
    else:
        from concourse.bass_utils import run_bass_kernel_spmd
        res = run_bass_kernel_spmd(nc, in_maps, core_ids=list(range(NC)),
                                   trace=trace)
        outs = [res.results[c]["out"] for c in range(NC)]
    full = np.empty((cfg["N"], cfg["COUT"]), np.float32)
    for c in range(NC):
        order2 = per_core[c]["order2_lo"]
        real = order2 < NLOC
        full[c * NLOC + order2[real]] = outs[c].T[real]
    return full, res


def kernel(**inputs):
    out, _ = run(inputs, FULL_CFG, use_sim=False)
    return out


# revision 11
# speedup vs baseline: 3.1101x; 1.0262x over previous
"""Trainium2 Bass kernel for nn_BlockConv (PointNet-style GNN block), 8 cores.

Algebraic core: msg_e = concat(x_src, pos_src-pos_dst) @ W + b
  = A[src] - C[dst], with A = concat(x,pos)@W (per-node table) and
  C = pos@W[-3:] - b (per-dst, constant within a segment).
  segment_max over dst = (gather+max of A rows) - C[dst]. Memory-bound.

Distribution: dst-sharded; per-core edge gathers from a replicated fp16
A-table in HBM via 4-queue SWDGE dma_gather (descriptor-rate limited:
~3ns/row on 4 queues vs ~9 on one). Tables, stages, h are fp16. Host
pre-transposes x/pos so table builds are straight fp16 matmuls; dst
slots are degree-sorted per side (lo/hi source windows for int16 idx)
so gather pass k covers a slot prefix. Dead (degree-0) slots get an
exact v=0 via a 5th posm row carrying -BIG through the C-matmul (no
mask ops). Stats fuse into the v-subtract via tensor_tensor_reduce.
The AllGather ships pre-BN v1^T concurrently with the stats AllReduce;
conv2's build applies BN+relu on the fly and fuses h@W2a + pos@W2p into
one 131-contract matmul. Output is feature-major in lo2-slot order;
the host unpermutes.
"""
import sys
import numpy as np

if "/opt/trn_rl_repo" not in sys.path:
    sys.path.insert(0, "/opt/trn_rl_repo")

BIG_NEG = -60000.0
EPS = 1e-5

FULL_CFG = dict(N=50000, E=800000, CIN=64, COUT=128, NC=8,
                LO_LIM=32768, R=50432)
MINI_CFG = dict(N=2048, E=16384, CIN=64, COUT=128, NC=8,
                LO_LIM=1024, R=2432)
MID_CFG = dict(N=16384, E=262144, CIN=64, COUT=128, NC=8,
               LO_LIM=8192, R=16768)


def _ceil(a, b):
    return (a + b - 1) // b


def _wrap16(ids):
    """flat int list (len % 128 == 0) -> [128, len//16] int16 wrapped:
    unwrapped[j] = g[j%16, j//16], replicated over the 8 core groups."""
    a = np.asarray(ids, np.int64)
    assert a.size % 128 == 0 and a.min() >= 0 and a.max() < 32768
    g = a.reshape(a.size // 16, 16).T.astype(np.int16)   # [16, L/16]
    return np.tile(g, (8, 1))                            # [128, L/16]


def _side_prep(rows, d_loc, NSLOT):
    """Degree-sort dst slots for one source window. rows = side-local table
    row ids per edge; d_loc = local dst node position per edge."""
    deg = np.bincount(d_loc, minlength=NSLOT)
    order = np.argsort(-deg, kind="stable")        # slot s -> node position
    slot_of = np.empty(NSLOT, np.int64)            # node position -> slot
    slot_of[order] = np.arange(NSLOT)
    isort = np.argsort(d_loc, kind="stable")
    starts = np.zeros(NSLOT + 1, np.int64)
    np.cumsum(deg, out=starts[1:])
    return {"deg": deg, "order": order, "slot_of": slot_of,
            "s_sorted": rows[isort], "starts": starts,
            "cnts": np.sort(deg)[::-1]}


def _build_sched(sides_by_core):
    """sched[si] = list of S_k (128-slot chunks) per pass, max across cores."""
    sched = []
    for si in range(2):
        Sk = []
        kmax = max(int(sd[si]["cnts"][0]) for sd in sides_by_core)
        for k in range(kmax):
            cnt = max(int((sd[si]["cnts"] > k).sum()) for sd in sides_by_core)
            if cnt == 0:
                break
            Sk.append(_ceil(cnt, 128))
        sched.append(Sk)
    return sched


def _build_gi(side, sched_side, dummy):
    blocks = []
    for k, S in enumerate(sched_side):
        L = S * 128
        ids = np.full(L, dummy, np.int64)
        nsl = int((side["cnts"] > k).sum())
        nodes = side["order"][:nsl]
        ids[:nsl] = side["s_sorted"][side["starts"][nodes] + k]
        blocks.append(_wrap16(ids))
    if not blocks:
        return np.zeros((128, 8), np.int16)
    return np.concatenate(blocks, axis=1)


def _posm_aug(pos, node, dead):
    """[5, NSLOT] fp16: rows 0-2 pos, row 3 = -1 (bias), row 4 = -BIG flag.
    Dead slots get rows 0-3 zeroed and row4=-BIG so C' = -BIG exactly and
    v = agg - C' = 0 exactly."""
    m = np.empty((5, node.size), np.float16)
    m[:3] = pos[node].T
    m[3] = -1.0
    m[4] = 0.0
    m[0:4, dead] = 0.0
    m[4, dead] = BIG_NEG
    return np.ascontiguousarray(m)


def host_prep(edge_index, pos, x, cfg):
    N, NC, LO_LIM, R = cfg["N"], cfg["NC"], cfg["LO_LIM"], cfg["R"]
    NLOC = N // NC
    SLOC = _ceil(NLOC, 128)
    NSLOT = SLOC * 128
    NPAD1 = _ceil(N, 2048) * 2048
    HID = R - 1 - LO_LIM          # hi-local dummy row (global row R-1)
    src = np.asarray(edge_index[0], np.int64)
    dst = np.asarray(edge_index[1], np.int64)
    core_of = dst // NLOC
    pos = np.asarray(pos, np.float32)
    x = np.asarray(x, np.float32)

    # ---- conv1: per-core lo/hi sides in source-node space ----
    data1, slot1_lo_glob = [], np.empty(N, np.int64)
    for c in range(NC):
        m = core_of == c
        r = src[m] + 1
        d = dst[m] - c * NLOC
        sides = []
        for si, sel in ((0, r < LO_LIM), (1, r >= LO_LIM)):
            sides.append(_side_prep(r[sel] - (LO_LIM if si else 0),
                                    d[sel], NSLOT))
        data1.append(sides)
        slot1_lo_glob[c * NLOC:(c + 1) * NLOC] = sides[0]["slot_of"][:NLOC]
    sched1 = _build_sched(data1)

    # ---- conv2: rows in (core, lo1-slot) space ----
    r2 = 1 + (src // NLOC) * NSLOT + slot1_lo_glob[src]
    data2 = []
    for c in range(NC):
        m = core_of == c
        r = r2[m]
        d = dst[m] - c * NLOC
        sides = []
        for si, sel in ((0, r < LO_LIM), (1, r >= LO_LIM)):
            sides.append(_side_prep(r[sel] - (LO_LIM if si else 0),
                                    d[sel], NSLOT))
        data2.append(sides)
    sched2 = _build_sched(data2)

    # ---- per-core tensors ----
    per_core = []
    for c in range(NC):
        s1, s2 = data1[c], data2[c]
        deg_tot = s1[0]["deg"] + s1[1]["deg"]      # per node position
        node1 = np.minimum(c * NLOC + s1[0]["order"], N - 1)
        node2 = np.minimum(c * NLOC + s2[0]["order"], N - 1)
        xs2 = np.zeros((64, NSLOT), np.float16)
        real2 = s2[0]["order"] < NLOC
        xs2[:, real2] = x[c * NLOC + s2[0]["order"][real2]].T
        per_core.append({
            "gi1_lo": _build_gi(s1[0], sched1[0], 0),
            "gi1_hi": _build_gi(s1[1], sched1[1], HID),
            "gi2_lo": _build_gi(s2[0], sched2[0], 0),
            "gi2_hi": _build_gi(s2[1], sched2[1], HID),
            "mg_hi1": _wrap16(s1[1]["slot_of"][s1[0]["order"]]),
            "mg_hi2": _wrap16(s2[1]["slot_of"][s2[0]["order"]]),
            "posm1": _posm_aug(pos, node1, deg_tot[s1[0]["order"]] == 0),
            "posm2": _posm_aug(pos, node2, deg_tot[s2[0]["order"]] == 0),
            "xsT2": np.ascontiguousarray(xs2),
            "order2_lo": s2[0]["order"],
        })

    # ---- shared tensors ----
    xpT = np.zeros((cfg["CIN"] + 3, NPAD1), np.float16)
    xpT[:cfg["CIN"], :N] = x.T
    xpT[cfg["CIN"]:, :N] = pos.T
    posw = np.empty((3, NC * NSLOT), np.float32)
    for c in range(NC):
        node1 = np.minimum(c * NLOC + data1[c][0]["order"], N - 1)
        posw[:, c * NSLOT:(c + 1) * NSLOT] = pos[node1].T
    shared = {"xpT": np.ascontiguousarray(xpT), "posw": posw}
    return per_core, shared, (sched1, sched2)


def build_bass(cfg, scheds, reps=1):
    import concourse.bass as bass
    import concourse.bacc as bacc
    import concourse.tile as tile
    from concourse import mybir
    from concourse.masks import make_identity
    import contextlib

    N, NC = cfg["N"], cfg["NC"]
    CIN, COUT = cfg["CIN"], cfg["COUT"]
    NLOC = N // NC
    SLOC = _ceil(NLOC, 128)
    NSLOT = SLOC * 128
    LO_LIM, R = cfg["LO_LIM"], cfg["R"]
    HI_R = R - LO_LIM
    NPAD1 = _ceil(N, 2048) * 2048
    sched1, sched2 = scheds
    f32, f16, i16 = mybir.dt.float32, mybir.dt.float16, mybir.dt.int16
    OP = mybir.AluOpType
    AF = mybir.ActivationFunctionType
    AXX = mybir.AxisListType.X
    VC = min(512, NSLOT)          # v-compute chunk (slots)

    nc = bacc.Bacc(num_devices=NC, name="blockconv3", num_swdge_queues=4)

    xpT_in = nc.dram_tensor("xpT", [CIN + 3, NPAD1], f16, kind="ExternalInput")
    poswP_in = nc.dram_tensor("poswP", [COUT, NC * NSLOT], f16,
                              kind="ExternalInput")
    xsT2_in = nc.dram_tensor("xsT2", [CIN, NSLOT], f16, kind="ExternalInput")
    posm1_in = nc.dram_tensor("posm1", [5, NSLOT], f16, kind="ExternalInput")
    posm2_in = nc.dram_tensor("posm2", [5, NSLOT], f16, kind="ExternalInput")
    wt = {}
    for nm, shp, dt in (("W1s", [CIN + 3, COUT], f16), ("W1pa", [5, COUT], f16),
                        ("W2as", [COUT, COUT], f16), ("W2pa", [5, COUT], f16),
                        ("Wls", [CIN, COUT], f16),
                        ("g1", [COUT, 1], f32), ("be1", [COUT, 1], f32),
                        ("g2", [COUT, 1], f32), ("be2", [COUT, 1], f32),
                        ("gl", [COUT, 1], f32), ("bel", [COUT, 1], f32)):
        wt[nm] = nc.dram_tensor(nm, shp, dt, kind="ExternalInput")

    W1lo = max(sum(sched1[0]), 1) * 8
    W1hi = max(sum(sched1[1]), 1) * 8
    W2lo = max(sum(sched2[0]), 1) * 8
    W2hi = max(sum(sched2[1]), 1) * 8
    gi_in = {}
    for nm, w in (("gi1_lo", W1lo), ("gi1_hi", W1hi),
                  ("gi2_lo", W2lo), ("gi2_hi", W2hi)):
        gi_in[nm] = nc.dram_tensor(nm, [128, w], i16, kind="ExternalInput")
    mg1_in = nc.dram_tensor("mg_hi1", [128, NSLOT // 16], i16, kind="ExternalInput")
    mg2_in = nc.dram_tensor("mg_hi2", [128, NSLOT // 16], i16, kind="ExternalInput")

    out_t = nc.dram_tensor("out", [COUT, NSLOT], f32, kind="ExternalOutput")

    table_lo = nc.dram_tensor("table_lo", [LO_LIM, COUT], f16)
    table_hi = nc.dram_tensor("table_hi", [HI_R, COUT], f16)
    mbuf = nc.dram_tensor("mbuf", [NSLOT, COUT], f16)
    ag_i = nc.dram_tensor("ag_in", [COUT, NSLOT], f16)
    ag_o = nc.dram_tensor("ag_out", [NC, COUT, NSLOT], f16, addr_space="Shared")
    ar_i = nc.dram_tensor("ar_in", [COUT, 4], f32)
    ar_o = nc.dram_tensor("ar_out", [COUT, 4], f32, addr_space="Shared")
    ar2_i = nc.dram_tensor("ar2_in", [COUT, 2], f32)
    ar2_o = nc.dram_tensor("ar2_out", [COUT, 2], f32, addr_space="Shared")
    bar_i = nc.dram_tensor("bar_in", [COUT, 1], f32)
    bar_o = nc.dram_tensor("bar_out", [COUT, 1], f32, addr_space="Shared")
    groups = [list(range(NC))]

    qctr = [0]

    def nextq():
        q = qctr[0] & 3
        qctr[0] += 1
        return q

    with tile.TileContext(nc) as tc:
        ctx = contextlib.ExitStack()
        with ctx:
            sing = ctx.enter_context(tc.tile_pool(name="sing", bufs=1))
            ld = ctx.enter_context(tc.tile_pool(name="ld", bufs=2))
            st = ctx.enter_context(tc.tile_pool(name="st", bufs=2))
            big = ctx.enter_context(tc.tile_pool(name="big", bufs=1))
            ck = ctx.enter_context(tc.tile_pool(name="ck", bufs=2))
            ppb = ctx.enter_context(tc.tile_pool(name="ppb", bufs=4, space="PSUM"))
            pcs = ctx.enter_context(tc.tile_pool(name="pcs", bufs=2, space="PSUM"))
            ppt = ctx.enter_context(tc.tile_pool(name="ppt", bufs=2, space="PSUM"))

            ident = sing.tile([128, 128], f16)
            make_identity(nc, ident)
            negbig = sing.tile([1, COUT], f16)
            nc.vector.memset(negbig[:], BIG_NEG)
            epsv = sing.tile([COUT, 1], f32)
            nc.vector.memset(epsv[:], EPS)
            zeros = sing.tile([128, VC], f32)
            nc.vector.memset(zeros[:], 0.0)

            W = {}
            for nm in ("W1s", "W1pa", "W2as", "W2pa", "Wls"):
                t = sing.tile(list(wt[nm].shape), f16, tag=f"w_{nm}")
                nc.sync.dma_start(t[:], wt[nm][:])
                W[nm] = t
            pvec = {}
            for nm in ("g1", "be1", "g2", "be2", "gl", "bel"):
                v = sing.tile([COUT, 1], f32, tag=f"pv_{nm}")
                nc.sync.dma_start(v[:], wt[nm][:])
                pvec[nm] = v

            mg1 = sing.tile([128, NSLOT // 16], i16, tag="mg1")
            nc.sync.dma_start(mg1[:], mg1_in[:])
            mg2 = sing.tile([128, NSLOT // 16], i16, tag="mg2")
            nc.sync.dma_start(mg2[:], mg2_in[:])

            def table_write(oc, base, G):
                """Write oc[:, 0:G, :] (G*128 rows) at table row `base`."""
                n = G * 128
                m0 = max(0, min(LO_LIM - base, n))
                if m0 > 0:
                    gf, rem = divmod(m0, 128)
                    if gf:
                        d = table_lo[base:base + gf * 128, :].rearrange(
                            "(g p) f -> p g f", p=128)
                        nc.sync.dma_start(d, oc[:, 0:gf, :])
                    if rem:
                        nc.sync.dma_start(
                            table_lo[base + gf * 128:base + m0, :],
                            oc[0:rem, gf, :])
                if m0 < n:
                    b2 = base + m0 - LO_LIM
                    gf, rem = divmod(m0, 128)
                    if rem:
                        nc.sync.dma_start(
                            table_hi[b2:b2 + (128 - rem), :],
                            oc[rem:128, gf, :])
                        b2 += 128 - rem
                        gf += 1
                    if gf < G:
                        d = table_hi[b2:b2 + (G - gf) * 128, :].rearrange(
                            "(g p) f -> p g f", p=128)
                        nc.sync.dma_start(d, oc[:, gf:G, :])

            def gathers(gi, sched_c, acc):
                """Max-accumulate gather passes for one conv into acc
                [128, 2*SLOC, COUT] f16 (lo ranks then hi ranks)."""
                for si, (win, winsz) in enumerate(
                        ((table_lo, LO_LIM), (table_hi, HI_R))):
                    idxt, off = gi[si], 0
                    for k, S in enumerate(sched_c[si]):
                        stg = st.tile([128, SLOC, COUT], f16, tag="stage")
                        a = 0
                        while a < S:
                            b = min(a + 8, S)
                            nc.gpsimd.dma_gather(
                                out_ap=stg[:, a:b, :], in_ap=win[0:winsz, :],
                                idxs_ap=idxt[:, off + a * 8:off + b * 8],
                                num_idxs=(b - a) * 128,
                                num_idxs_reg=(b - a) * 128,
                                elem_size=COUT, queue_num=nextq())
                            a = b
                        ro = si * SLOC
                        nc.vector.tensor_tensor(
                            out=acc[:, ro:ro + S, :], in0=acc[:, ro:ro + S, :],
                            in1=stg[:, 0:S, :], op=OP.max)
                        off += S * 8

            def merge_and_v(acc, mg, posm_t, Wp_aug, vT):
                """acc -> agg (lo order) -> transpose -> v^T = agg^T - C'.
                Fused running sum/sumsq via tensor_tensor_reduce."""
                nc.sync.dma_start(
                    mbuf[:].rearrange("(s p) f -> p s f", p=128),
                    acc[:, SLOC:2 * SLOC, :])
                hi_lo = st.tile([128, SLOC, COUT], f16, tag="stage")
                a = 0
                while a < SLOC:
                    b = min(a + 8, SLOC)
                    nc.gpsimd.dma_gather(
                        out_ap=hi_lo[:, a:b, :], in_ap=mbuf[:, :],
                        idxs_ap=mg[:, a * 8:b * 8],
                        num_idxs=(b - a) * 128, num_idxs_reg=(b - a) * 128,
                        elem_size=COUT, queue_num=nextq())
                    a = b
                nc.vector.tensor_tensor(out=hi_lo[:], in0=hi_lo[:],
                                        in1=acc[:, 0:SLOC, :], op=OP.max)
                ssum = [None, None]   # ping-pong [COUT,1] accumulators
                sqq = [None, None]
                nchunk = _ceil(NSLOT, VC)
                for ci in range(nchunk):
                    j0 = ci * VC
                    nr = min(VC, NSLOT - j0) // 128
                    w = nr * 128
                    pt = ppt.tile([128, VC // 128, 128], f16, tag="pt")
                    for r in range(nr):
                        nc.tensor.transpose(out=pt[:, r, :],
                                            in_=hi_lo[:, j0 // 128 + r, :],
                                            identity=ident[:])
                    ptf = pt[:, 0:nr, :].rearrange("p a b -> p (a b)")
                    aggf = ck.tile([128, VC], f32, tag="aggf")
                    nc.scalar.copy(out=aggf[:, 0:w], in_=ptf)
                    cps = pcs.tile([128, VC], f32, tag="cps")
                    nc.tensor.matmul(out=cps[:, 0:w], lhsT=Wp_aug[:],
                                     rhs=posm_t[:, j0:j0 + w],
                                     start=True, stop=True)
                    nc.vector.tensor_tensor(out=vT[:, j0:j0 + w],
                                            in0=aggf[:, 0:w], in1=cps[:, 0:w],
                                            op=OP.subtract)
                    ps = ck.tile([COUT, 1], f32, tag="ps")
                    nc.vector.tensor_reduce(out=ps[:], in_=vT[:, j0:j0 + w],
                                            op=OP.add, axis=AXX)
                    junk = ck.tile([128, VC], f32, tag="junk")
                    nc.vector.tensor_tensor(out=junk[:, 0:w],
                                            in0=vT[:, j0:j0 + w],
                                            in1=vT[:, j0:j0 + w], op=OP.mult)
                    pq = ck.tile([COUT, 1], f32, tag="pq")
                    nc.vector.tensor_reduce(out=pq[:], in_=junk[:, 0:w],
                                            op=OP.add, axis=AXX)
                    cs = ck.tile([COUT, 1], f32, tag=f"ms{ci & 1}")
                    cq = ck.tile([COUT, 1], f32, tag=f"mq{ci & 1}")
                    if ci == 0:
                        nc.vector.tensor_copy(out=cs[:], in_=ps[:])
                        nc.vector.tensor_copy(out=cq[:], in_=pq[:])
                    else:
                        nc.vector.tensor_tensor(out=cs[:], in0=ps[:],
                                                in1=ssum[(ci - 1) & 1][:],
                                                op=OP.add)
                        nc.vector.tensor_tensor(out=cq[:], in0=pq[:],
                                                in1=sqq[(ci - 1) & 1][:],
                                                op=OP.add)
                    ssum[ci & 1] = cs
                    sqq[ci & 1] = cq
                return ssum[(nchunk - 1) & 1], sqq[(nchunk - 1) & 1]

            def bn_params(sum_ap, sq_ap, g_v, be_v, tagp):
                """scale = g*rsqrt(var+eps), shift = be - mean*scale; [COUT,1]."""
                mean = ck.tile([COUT, 1], f32, tag=f"{tagp}_m")
                nc.vector.tensor_scalar(out=mean[:], in0=sum_ap, scalar1=1.0 / N,
                                        scalar2=None, op0=OP.mult)
                ex2 = ck.tile([COUT, 1], f32, tag=f"{tagp}_e")
                nc.vector.tensor_scalar(out=ex2[:], in0=sq_ap, scalar1=1.0 / N,
                                        scalar2=None, op0=OP.mult)
                m2 = ck.tile([COUT, 1], f32, tag=f"{tagp}_m2")
                nc.vector.tensor_tensor(out=m2[:], in0=mean[:], in1=mean[:],
                                        op=OP.mult)
                var = ck.tile([COUT, 1], f32, tag=f"{tagp}_v")
                nc.vector.tensor_tensor(out=var[:], in0=ex2[:], in1=m2[:],
                                        op=OP.subtract)
                sd = ck.tile([COUT, 1], f32, tag=f"{tagp}_sd")
                nc.scalar.activation(out=sd[:], in_=var[:], func=AF.Sqrt,
                                     bias=epsv[:], scale=1.0)
                rstd = ck.tile([COUT, 1], f32, tag=f"{tagp}_r")
                nc.vector.reciprocal(out=rstd[:], in_=sd[:])
                sc = sing.tile([COUT, 1], f32, tag=f"{tagp}_sc")
                nc.vector.tensor_tensor(out=sc[:], in0=rstd[:], in1=g_v[:],
                                        op=OP.mult)
                ms = ck.tile([COUT, 1], f32, tag=f"{tagp}_ms")
                nc.vector.tensor_tensor(out=ms[:], in0=mean[:], in1=sc[:],
                                        op=OP.mult)
                sh = sing.tile([COUT, 1], f32, tag=f"{tagp}_sh")
                nc.vector.tensor_tensor(out=sh[:], in0=be_v[:], in1=ms[:],
                                        op=OP.subtract)
                return sc, sh

            for _rep in range(reps):
                if _rep == 0:
                    # pre-barrier: overlap cross-core rendezvous with conv1 build
                    nc.sync.dma_start(bar_i[:], epsv[:])
                    nc.gpsimd.collective_compute(
                        "AllReduce", OP.add, replica_groups=groups,
                        ins=[bar_i[:]], outs=[bar_o[:]])

                # ---- dummy rows ----
                nc.sync.dma_start(table_lo[0:1, :], negbig[:])
                nc.sync.dma_start(table_hi[HI_R - 1:HI_R, :], negbig[:])

                # ---- gather index loads (early; sync is idle later) ----
                gi1, gi2 = {}, {}
                for d, pre in ((gi1, "gi1"), (gi2, "gi2")):
                    for si, side in ((0, "lo"), (1, "hi")):
                        nm = f"{pre}_{side}"
                        t = sing.tile([128, gi_in[nm].shape[1]], i16,
                                      tag=f"t_{nm}")
                        nc.sync.dma_start(t[:], gi_in[nm][:])
                        d[si] = t

                # ---- conv1 A-table build ----
                for c0 in range(0, NPAD1, 2048):
                    lhs = ld.tile([CIN + 3, 2048], f16, tag="lhs1")
                    nc.sync.dma_start(lhs[:], xpT_in[:, c0:c0 + 2048])
                    oc = ld.tile([128, 16, COUT], f16, tag="oc")
                    for g2_ in range(8):
                        pb = ppb.tile([128, 2, COUT], f32, tag="pb")
                        for h in range(2):
                            nc.tensor.matmul(
                                out=pb[:, h, :],
                                lhsT=lhs[:, (g2_ * 2 + h) * 128:
                                         (g2_ * 2 + h + 1) * 128],
                                rhs=W["W1s"][:], start=True, stop=True)
                        nc.scalar.copy(out=oc[:, g2_ * 2:g2_ * 2 + 2, :],
                                       in_=pb[:])
                    table_write(oc, 1 + c0, min(16, _ceil(N - c0, 128)))

                # ---- conv1 gathers ----
                acc = big.tile([128, 2 * SLOC, COUT], f16, tag="acc")
                nc.vector.memset(acc[:], BIG_NEG)
                gathers(gi1, sched1, acc)

                # ---- skip path: skipT = Wl^T @ xsT2 (lo2 order) ----
                skipT = big.tile([COUT, NSLOT], f16, tag="skipT")
                sks = [None, None]
                skq = [None, None]
                nsk = _ceil(NSLOT, VC)
                for ci in range(nsk):
                    j0 = ci * VC
                    hw = min(VC, NSLOT - j0)
                    xs = ld.tile([CIN, VC], f16, tag="lhs2")
                    nc.sync.dma_start(xs[:, 0:hw], xsT2_in[:, j0:j0 + hw])
                    pskip = pcs.tile([128, VC], f32, tag="cps")
                    nc.tensor.matmul(out=pskip[:, 0:hw], lhsT=W["Wls"][:],
                                     rhs=xs[:, 0:hw], start=True, stop=True)
                    nc.scalar.copy(out=skipT[:, j0:j0 + hw],
                                   in_=pskip[:, 0:hw])
                    ps = ck.tile([COUT, 1], f32, tag="ps")
                    nc.vector.tensor_reduce(out=ps[:],
                                            in_=skipT[:, j0:j0 + hw],
                                            op=OP.add, axis=AXX)
                    junk = ck.tile([128, VC], f32, tag="junk")
                    nc.vector.tensor_tensor(out=junk[:, 0:hw],
                                            in0=skipT[:, j0:j0 + hw],
                                            in1=skipT[:, j0:j0 + hw],
                                            op=OP.mult)
                    pq = ck.tile([COUT, 1], f32, tag="pq")
                    nc.vector.tensor_reduce(out=pq[:], in_=junk[:, 0:hw],
                                            op=OP.add, axis=AXX)
                    cs = ck.tile([COUT, 1], f32, tag=f"ss{ci & 1}")
                    cq = ck.tile([COUT, 1], f32, tag=f"sq{ci & 1}")
                    if ci == 0:
                        nc.vector.tensor_copy(out=cs[:], in_=ps[:])
                        nc.vector.tensor_copy(out=cq[:], in_=pq[:])
                    else:
                        nc.vector.tensor_tensor(out=cs[:], in0=ps[:],
                                                in1=sks[(ci - 1) & 1][:],
                                                op=OP.add)
                        nc.vector.tensor_tensor(out=cq[:], in0=pq[:],
                                                in1=skq[(ci - 1) & 1][:],
                                                op=OP.add)
                    sks[ci & 1] = cs
                    skq[ci & 1] = cq
                sksum, sksq = sks[(nsk - 1) & 1], skq[(nsk - 1) & 1]

                # ---- conv1 merge + v1 + stats ----
                posm1 = sing.tile([5, NSLOT], f16, tag="posm")
                nc.sync.dma_start(posm1[:], posm1_in[:])
                v1T = big.tile([COUT, NSLOT], f16, tag="vT")
                s1, q1 = merge_and_v(acc, mg1, posm1, W["W1pa"], v1T)

                # AllGather of pre-BN v1^T runs concurrently with the
                # stats AllReduce; BN1+relu applied in conv2's build.
                nc.sync.dma_start(ag_i[:], v1T[:])
                nc.gpsimd.collective_compute(
                    "AllGather", OP.bypass, replica_groups=groups,
                    ins=[ag_i[:]], outs=[ag_o[:]])

                arst = sing.tile([COUT, 4], f32, tag="arst")
                nc.vector.tensor_copy(out=arst[:, 0:1], in_=s1[:])
                nc.vector.tensor_copy(out=arst[:, 1:2], in_=q1[:])
                nc.vector.tensor_copy(out=arst[:, 2:3], in_=sksum[:])
                nc.vector.tensor_copy(out=arst[:, 3:4], in_=sksq[:])
                nc.sync.dma_start(ar_i[:], arst[:])
                nc.gpsimd.collective_compute(
                    "AllReduce", OP.add, replica_groups=groups,
                    ins=[ar_i[:]], outs=[ar_o[:]])
                arres = sing.tile([COUT, 4], f32, tag="arres")
                nc.sync.dma_start(arres[:], ar_o[:])

                sc1, sh1 = bn_params(arres[:, 0:1], arres[:, 1:2],
                                     pvec["g1"], pvec["be1"], "bn1")
                scl, shl = bn_params(arres[:, 2:3], arres[:, 3:4],
                                     pvec["gl"], pvec["bel"], "bnl")

                # ---- conv2 A-table build (fused 131-contract matmul) ----
                for ct in range(NC):
                    for c0 in range(0, NSLOT, 1024):
                        w = min(1024, NSLOT - c0)
                        nw = w // 128
                        vstg = ld.tile([COUT, 1024], f16, tag="vstg")
                        nc.sync.dma_start(vstg[:, 0:w], ag_o[ct, :, c0:c0 + w])
                        pwp = ld.tile([COUT, 1024], f16, tag="pwp")
                        nc.sync.dma_start(
                            pwp[:, 0:w],
                            poswP_in[:, ct * NSLOT + c0:ct * NSLOT + c0 + w])
                        lhs = ld.tile([COUT, 1024], f16, tag="lhs2b")
                        nc.scalar.activation(out=lhs[:, 0:w],
                                             in_=vstg[:, 0:w], func=AF.Relu,
                                             bias=sh1[:], scale=sc1[:])
                        nc.vector.tensor_tensor(out=lhs[:, 0:w],
                                                in0=lhs[:, 0:w],
                                                in1=pwp[:, 0:w], op=OP.add)
                        oc = ld.tile([128, 8, COUT], f16, tag="oc2")
                        for g2_ in range(_ceil(nw, 2)):
                            pb = ppb.tile([128, 2, COUT], f32, tag="pb")
                            for h in range(min(2, nw - g2_ * 2)):
                                g = g2_ * 2 + h
                                nc.tensor.matmul(
                                    out=pb[:, h, :],
                                    lhsT=lhs[:, g * 128:(g + 1) * 128],
                                    rhs=W["W2as"][:], start=True, stop=True)
                            nh = min(2, nw - g2_ * 2)
                            nc.scalar.copy(out=oc[:, g2_ * 2:g2_ * 2 + nh, :],
                                           in_=pb[:, 0:nh, :])
                        table_write(oc, 1 + ct * NSLOT + c0, nw)

                # ---- conv2 gathers ----
                acc2 = big.tile([128, 2 * SLOC, COUT], f16, tag="acc")
                nc.vector.memset(acc2[:], BIG_NEG)
                gathers(gi2, sched2, acc2)

                # ---- conv2 merge + v2 + stats ----
                posm2 = sing.tile([5, NSLOT], f16, tag="posm")
                nc.sync.dma_start(posm2[:], posm2_in[:])
                v2T = big.tile([COUT, NSLOT], f16, tag="vT")
                s2, q2 = merge_and_v(acc2, mg2, posm2, W["W2pa"], v2T)

                arst2 = sing.tile([COUT, 2], f32, tag="arst2")
                nc.vector.tensor_copy(out=arst2[:, 0:1], in_=s2[:])
                nc.vector.tensor_copy(out=arst2[:, 1:2], in_=q2[:])
                nc.sync.dma_start(ar2_i[:], arst2[:])
                nc.gpsimd.collective_compute(
                    "AllReduce", OP.add, replica_groups=groups,
                    ins=[ar2_i[:]], outs=[ar2_o[:]])
                arres2 = sing.tile([COUT, 2], f32, tag="arres2")
                nc.sync.dma_start(arres2[:], ar2_o[:])
                sc2, sh2 = bn_params(arres2[:, 0:1], arres2[:, 1:2],
                                     pvec["g2"], pvec["be2"], "bn2")

                # ---- final = relu(bn2(v2) + bnl(skip)) ----
                for j0 in range(0, NSLOT, VC):
                    w = min(VC, NSLOT - j0)
                    a_ = ck.tile([128, VC], f32, tag="aggf")
                    nc.scalar.activation(out=a_[:, 0:w],
                                         in_=v2T[:, j0:j0 + w],
                                         func=AF.Identity,
                                         bias=sh2[:], scale=sc2[:])
                    b_ = ck.tile([128, VC], f32, tag="junk")
                    nc.scalar.activation(out=b_[:, 0:w],
                                         in_=skipT[:, j0:j0 + w],
                                         func=AF.Identity,
                                         bias=shl[:], scale=scl[:])
                    fin = ck.tile([128, VC], f32, tag="fin")
                    nc.vector.tensor_tensor(out=fin[:, 0:w], in0=a_[:, 0:w],
                                            in1=b_[:, 0:w], op=OP.add)
                    nc.vector.tensor_scalar(out=fin[:, 0:w], in0=fin[:, 0:w],
                                            scalar1=0.0, scalar2=None,
                                            op0=OP.max)
                    nc.sync.dma_start(out_t[:, j0:j0 + w], fin[:, 0:w])

    nc.compile()
    return nc


def make_in_maps(inputs, cfg, per_core, shared):
    f16 = np.float16
    CIN, COUT = cfg["CIN"], cfg["COUT"]
    W1 = np.asarray(inputs["W1"], np.float32)
    b1 = np.asarray(inputs["b1"], np.float32)
    W2 = np.asarray(inputs["W2"], np.float32)
    b2 = np.asarray(inputs["b2"], np.float32)
    ones = np.ones((1, COUT), np.float32)
    W1pa = np.concatenate([W1[CIN:CIN + 3, :], b1[None, :], ones], axis=0)
    W2pa = np.concatenate([W2[COUT:COUT + 3, :], b2[None, :], ones], axis=0)
    W2a = W2[0:COUT, :].astype(np.float64)
    W2p = W2[COUT:COUT + 3, :].astype(np.float64)
    P = np.linalg.solve(W2a.T, W2p.T).T          # P @ W2a == W2p
    poswP = (P.T @ shared["posw"].astype(np.float64)).astype(f16)
    base = dict(
        xpT=shared["xpT"], poswP=np.ascontiguousarray(poswP),
        W1s=W1.astype(f16), W1pa=W1pa.astype(f16),
        W2as=W2[0:COUT, :].astype(f16), W2pa=W2pa.astype(f16),
        Wls=np.asarray(inputs["Wl"], np.float32).astype(f16),
        g1=np.asarray(inputs["g1"], np.float32).reshape(-1, 1),
        be1=np.asarray(inputs["be1"], np.float32).reshape(-1, 1),
        g2=np.asarray(inputs["g2"], np.float32).reshape(-1, 1),
        be2=np.asarray(inputs["be2"], np.float32).reshape(-1, 1),
        gl=np.asarray(inputs["gl"], np.float32).reshape(-1, 1),
        bel=np.asarray(inputs["bel"], np.float32).reshape(-1, 1),
    )
    in_maps = []
    for pc in per_core:
        m = dict(base)
        for k in ("gi1_lo", "gi1_hi", "gi2_lo", "gi2_hi",
                  "mg_hi1", "mg_hi2", "posm1", "posm2", "xsT2"):
            m[k] = pc[k]
        in_maps.append(m)
    return in_maps


_CACHE = {}


def run(inputs, cfg, use_sim=False, trace=False):
    per_core, shared, scheds = host_prep(
        inputs["edge_index"], inputs["pos"], inputs["x"], cfg)
    key = (cfg["N"], tuple(scheds[0][0]), tuple(scheds[0][1]),
           tuple(scheds[1][0]), tuple(scheds[1][1]))
    if key not in _CACHE:
        _CACHE[key] = build_bass(cfg, scheds)
    nc = _CACHE[key]
    in_maps = make_in_maps(inputs, cfg, per_core, shared)
    NC = cfg["NC"]
    NLOC = cfg["N"] // NC
    if use_sim:
        from concourse.bass_interp import MultiCoreSim
        sim = MultiCoreSim(nc, num_cores=NC, require_finite=False,
                           require_nnan=False)
        for c in range(NC):
            for k, v in in_maps[c].items():
                sim.cores[c].tensor(k)[:] = v
        sim.simulate(check_with_hw=False)
        outs = [np.array(sim.cores[c].tensor("out")) for c in range(NC)]
        res = None
    else:
        from concourse.bass_utils import run_bass_kernel_spmd
        res = run_bass_kernel_spmd(nc, in_maps, core_ids=list(range(NC)),
                                   trace=trace)
        outs = [res.results[c]["out"] for c in range(NC)]
    full = np.empty((cfg["N"], cfg["COUT"]), np.float32)
    for c in range(NC):
        order2 = per_core[c]["order2_lo"]
        real = order2 < NLOC
        full[c * NLOC + order2[real]] = outs[c].T[real]
    return full, res


def kernel(**inputs):
    out, _ = run(inputs, FULL_CFG, use_sim=False)
    return out
